# revision 1
# baseline (speedup 1.0000x reference)
"""DepthConsistencyLoss Trainium2 kernel (8 NeuronCores, batch-parallel).

loss = mean_{n,l} sum_{r=0..188} w_{r%9}[l] * (cam_unfold[r,l] - cam_center[r%21,l])^2

Restructured (verified exactly against the reference):
  loss*N*H*W = sum_n ( term1 - 2*term2 + term3 ) with, per batch element n:
    term1 = sum_p sum_l w_p * S_{dp}(E)        E = sum_c cam_c^2
    term2 = sum_g sum_{p in g} sum_l w_p * S_{dp}(Pi_g)
            Pi_g = sum_{c0} P_c0 * S_{(dy,0)}(cam_{c'})   (13 distinct products)
            P_c0 = cam_c0 + cam_{c0+7} + cam_{c0+14}
    term3 = 3 * sum_{c'} sum_l G_c' * Omega_c'            G = cam^2
            Omega from 9 shifted partial sums of wsum_m = w_m+w_{m+3}+w_{m+6}
  w_p = wspat_p * exp(-50*(S_{dp}(D) - D)^2), w_4 == 1.

Layout: partitions = 112 image rows per y-tile (2 tiles), free dim = [img][x]
(x padded 2+2 -> 228). Compute APs always start at partition 0 (HW rule:
start must be 0/32/64/96). All y-shifted operands are DMA-materialized
version buffers (partition-offset DMA is legal), with cross-tile slivers
for rows crossing the tile boundary and zero padding at image edges.
x-shifts are free-dim offsets.

Each core handles one batch element; host sums the 8 x [2,112,24] partials.
"""

import os
import sys

import numpy as np

for _p in ("/opt/trn_rl_repo", os.path.expanduser("~/.axon_site/_ro/trn_rl_repo")):
    if os.path.isdir(_p) and _p not in sys.path:
        sys.path.insert(0, _p)

import concourse.bass as bass
import concourse.bacc as bacc
import concourse.tile as tile
from concourse import mybir
from concourse.bass_utils import run_bass_kernel_spmd

F32 = mybir.dt.float32
BF16 = mybir.dt.bfloat16
Alu = mybir.AluOpType
Act = mybir.ActivationFunctionType

N, C, H, W = 8, 21, 224, 224
XF = 228
X0, X1 = 2, 226
NP = 112           # partitions per y-tile = core rows
NACC = 24
SIGMA_S = 5.0
STAGE = int(os.environ.get("DCL_STAGE", "4"))


def _delta(p):
    return (p // 3 - 1, p % 3 - 1)


def _cp_of_j(j):
    row = 84 + j
    return row // 9, row % 9


def _tables():
    table = {}
    for g in range(3):
        for c0 in range(7):
            ents = []
            for p in (3 * g, 3 * g + 1, 3 * g + 2):
                j = (9 * c0 + p) % 21
                cpr, ppr = _cp_of_j(j)
                dpy, dpx = _delta(p)
                dqy, dqx = _delta(ppr)
                ents.append((cpr, dqy - dpy, dqx - dpx))
            assert ents[0] == ents[1] == ents[2], (g, c0, ents)
            cpr, dy, dx = ents[0]
            assert dx == 0
            table[(g, c0)] = (cpr, dy)
    prods = sorted({(dy, c0, cpr) for (g, c0), (cpr, dy) in table.items()})
    pidx = {pr: i for i, pr in enumerate(prods)}
    groups = {
        g: [pidx[(table[(g, c0)][1], c0, table[(g, c0)][0])] for c0 in range(7)]
        for g in range(3)
    }
    return table, prods, groups


def _prod_runs(prods):
    runs = []
    for i, (dy, c0, cpr) in enumerate(prods):
        if runs and runs[-1][0] == dy and c0 == runs[-1][1] + runs[-1][3] \
                and cpr == runs[-1][2] + runs[-1][3]:
            runs[-1][3] += 1
        else:
            runs.append([dy, c0, cpr, 1, i])
    return runs


def _wspat():
    d2 = np.array([(p // 3 - 1) ** 2 + (p % 3 - 1) ** 2 for p in range(9)],
                  dtype=np.float64)
    return np.exp(-d2 / (2.0 * SIGMA_S ** 2))


class _TileCtx:
    """Per-y-tile buffer set."""

    def __init__(self, pool, t):
        self.t = t
        self.stg = pool.tile([NP, C, XF], F32, name=f"stg{t}", tag=f"stg{t}")
        self.dstg = pool.tile([NP, XF], F32, name=f"dstg{t}", tag=f"dstg{t}")
        self.ds = {d: pool.tile([NP, XF], F32, name=f"ds{d}_{t}", tag=f"ds{d}_{t}") for d in (-1, 1)}
        self.camb = pool.tile([NP, 3, XF], BF16, name=f"camb{t}", tag=f"camb{t}")
        self.cs = {d: pool.tile([NP, 3, XF], BF16, name=f"cs{d}_{t}", tag=f"cs{d}_{t}")
                   for d in (-2, -1, 1, 2)}
        self.gsq = pool.tile([NP, C, XF], BF16, name=f"gsq{t}", tag=f"gsq{t}")
        self.etr = pool.tile([NP, 20, XF], BF16, name=f"etr{t}", tag=f"etr{t}")
        self.eimg = pool.tile([NP, XF], BF16, name=f"eimg{t}", tag=f"eimg{t}")
        self.es = {d: pool.tile([NP, XF], BF16, name=f"es{d}_{t}", tag=f"es{d}_{t}") for d in (-1, 1)}
        self.Pb = pool.tile([NP, 7, XF], BF16, name=f"P{t}", tag=f"P{t}")
        self.prod21 = pool.tile([NP, 21, XF], BF16, name=f"prod21_{t}", tag=f"prod21_{t}")
        self.ptree = pool.tile([NP, 9, XF], BF16, name=f"ptree{t}", tag=f"ptree{t}")
        self.qbuf = pool.tile([NP, 3, XF], BF16, name=f"qbuf{t}", tag=f"qbuf{t}")
        self.Pi = pool.tile([NP, 3, XF], BF16, name=f"Pi{t}", tag=f"Pi{t}")
        self.pis = {d: pool.tile([NP, 3, XF], BF16, name=f"pis{d}_{t}", tag=f"pis{d}_{t}")
                    for d in (-1, 1)}
        self.wb = pool.tile([NP, 9, XF], BF16, name=f"w{t}", tag=f"w{t}")
        self.wsb = pool.tile([NP, 3, XF], BF16, name=f"ws{t}", tag=f"ws{t}")
        self.wss = {d: pool.tile([NP, 3, XF], BF16, name=f"wss{d}_{t}", tag=f"wss{d}_{t}")
                    for d in (-1, 1)}
        self.om = pool.tile([NP, 3, XF], BF16, name=f"om{t}", tag=f"om{t}")
        self.omt = pool.tile([NP, 3, XF], BF16, name=f"omt{t}", tag=f"omt{t}")
        self.ddif = pool.tile([NP, 8, XF], F32, name=f"ddif{t}", tag=f"ddif{t}")
        self.dsq = pool.tile([NP, 8, XF], F32, name=f"dsq{t}", tag=f"dsq{t}")
        self.scr = pool.tile([NP, 3, XF], BF16, name=f"scr{t}", tag=f"scr{t}")
        self.acc = pool.tile([NP, NACC], F32, name=f"acc{t}", tag=f"acc{t}")
        self.bias2 = pool.tile([NP, 2], F32, name=f"bias{t}", tag=f"bias{t}")


def _emit_shift(nc, tcs, t, dst, src_name, dy, nimg):
    """dst[p, ...] = global_src[112*t + p + dy, ...] with zero pad at image edges.

    src_name: attribute on _TileCtx holding the base image buffer (same shape
    as dst). dst must be pre-zeroed. Emits 1-2 DMAs (own part + neighbor sliver).
    """
    def src_of(tt):
        return getattr(tcs[tt], src_name)

    def sl(buf, p0, p1):
        return buf[p0:p1] if nimg == 1 else buf[p0:p1, :, :]

    # own-tile part: rows p with p+dy in [0, NP)
    p0, p1 = max(0, -dy), min(NP, NP - dy)
    nc.sync.dma_start(out=sl(dst, p0, p1), in_=sl(src_of(t), p0 + dy, p1 + dy))
    # neighbor sliver
    if dy > 0 and t == 0:       # rows [NP-dy, NP) come from tile1 rows [0, dy)
        nc.sync.dma_start(out=sl(dst, NP - dy, NP), in_=sl(src_of(1), 0, dy))
    if dy < 0 and t == 1:       # rows [0, -dy) come from tile0 rows [NP+dy, NP)
        nc.sync.dma_start(out=sl(dst, 0, -dy), in_=sl(src_of(0), NP + dy, NP))
    # image-edge rows stay zero (dst pre-memset)


def _emit_tile_pre(nc, tcs, t, cam, dep):
    """Stage 1: loads, conversions, squares, base images (no cross-tile deps)."""
    b = tcs[t]
    v = nc.vector
    s = nc.scalar
    wspat = _wspat()
    y0 = NP * t

    # DMA loads (per-channel; bacc's generate_event_semaphores handles the
    # consumer-side wait fan-in)
    for c in range(C):
        nc.sync.dma_start(out=b.stg[:, c, X0:X1], in_=cam[c, y0:y0 + NP, :])
    v.memset(b.dstg[:, :], 0.0)
    nc.sync.dma_start(out=b.dstg[:, X0:X1], in_=dep[0, y0:y0 + NP, :])

    # zero-init
    v.memset(b.acc[:, :], 0.0)
    v.memset(b.Pi[:, :, :], 0.0)
    v.memset(b.wsb[:, :, :], 0.0)
    v.memset(b.bias2[:, 0:1], float(np.log(wspat[0])))
    v.memset(b.bias2[:, 1:2], float(np.log(wspat[1])))

    # bf16 C channels
    s.activation(out=b.camb[:, :, X0:X1], in_=b.stg[:, 9:12, X0:X1],
                 func=Act.Copy)

    # squares (ACT), f32 in -> bf16 out
    s.activation(out=b.gsq[:, :, X0:X1], in_=b.stg[:, :, X0:X1], func=Act.Square)

    # E tree
    E = 19
    v.tensor_tensor(out=b.etr[:, 0:10, X0:X1], in0=b.gsq[:, 0:20:2, X0:X1],
                    in1=b.gsq[:, 1:20:2, X0:X1], op=Alu.add)
    v.tensor_tensor(out=b.etr[:, 10:15, X0:X1], in0=b.etr[:, 0:10:2, X0:X1],
                    in1=b.etr[:, 1:10:2, X0:X1], op=Alu.add)
    v.tensor_tensor(out=b.etr[:, 15:17, X0:X1], in0=b.etr[:, 10:14:2, X0:X1],
                    in1=b.etr[:, 11:14:2, X0:X1], op=Alu.add)
    v.tensor_tensor(out=b.etr[:, 17, X0:X1], in0=b.etr[:, 15, X0:X1],
                    in1=b.etr[:, 16, X0:X1], op=Alu.add)
    v.tensor_tensor(out=b.etr[:, 18, X0:X1], in0=b.etr[:, 17, X0:X1],
                    in1=b.etr[:, 14, X0:X1], op=Alu.add)
    v.memset(b.eimg[:, :], 0.0)
    v.tensor_tensor(out=b.eimg[:, X0:X1], in0=b.etr[:, 18, X0:X1],
                    in1=b.gsq[:, 20, X0:X1], op=Alu.add)

    # P
    v.tensor_tensor(out=b.Pb[:, :, X0:X1], in0=b.stg[:, 0:7, X0:X1],
                    in1=b.stg[:, 7:14, X0:X1], op=Alu.add)
    v.tensor_tensor(out=b.Pb[:, :, X0:X1], in0=b.Pb[:, :, X0:X1],
                    in1=b.stg[:, 14:21, X0:X1], op=Alu.add)


def _emit_tile_main(nc, tcs, t, out):
    """Stage 2: shifted versions, products, weights, reductions."""
    if STAGE < 2:
        return
    b = tcs[t]
    v = nc.vector
    s = nc.scalar
    table, prods, groups = _tables()

    # shifted C versions (pure-y shifts for the 13 products)
    for d in (-2, -1, 1, 2):
        v.memset(b.cs[d][:, :, :], 0.0)
        _emit_shift(nc, tcs, t, b.cs[d], "camb", d, 3)

    # 21 products in group-major slots (runs of consecutive c0/c' per group)
    for g in range(3):
        c0 = 0
        while c0 < 7:
            cpr, dy = table[(g, c0)]
            n = 1
            while c0 + n < 7 and table[(g, c0 + n)] == (cpr + n, dy):
                n += 1
            srcb = b.camb if dy == 0 else b.cs[dy]
            v.tensor_tensor(out=b.prod21[:, 7 * g + c0:7 * g + c0 + n, X0:X1],
                            in0=b.Pb[:, c0:c0 + n, X0:X1],
                            in1=srcb[:, cpr - 9:cpr - 9 + n, X0:X1], op=Alu.mult)
            c0 += n
    # regular tree: 9 pair-adds, then 3+3+3
    P21, PT = b.prod21, b.ptree
    pst, tst = P21.ap[0][0], PT.ap[0][0]
    v.tensor_tensor(
        out=bass.AP(PT.tensor, PT.offset + X0,
                    [[tst, NP], [3 * XF, 3], [XF, 3], [1, 224]]),
        in0=bass.AP(P21.tensor, P21.offset + X0,
                    [[pst, NP], [7 * XF, 3], [2 * XF, 3], [1, 224]]),
        in1=bass.AP(P21.tensor, P21.offset + XF + X0,
                    [[pst, NP], [7 * XF, 3], [2 * XF, 3], [1, 224]]),
        op=Alu.add)
    v.tensor_tensor(
        out=b.qbuf[:, :, X0:X1],
        in0=bass.AP(PT.tensor, PT.offset + X0, [[tst, NP], [3 * XF, 3], [1, 224]]),
        in1=bass.AP(PT.tensor, PT.offset + XF + X0, [[tst, NP], [3 * XF, 3], [1, 224]]),
        op=Alu.add)
    v.tensor_tensor(
        out=b.qbuf[:, :, X0:X1], in0=b.qbuf[:, :, X0:X1],
        in1=bass.AP(PT.tensor, PT.offset + 2 * XF + X0,
                    [[tst, NP], [3 * XF, 3], [1, 224]]),
        op=Alu.add)
    v.tensor_tensor(
        out=b.Pi[:, :, X0:X1], in0=b.qbuf[:, :, X0:X1],
        in1=bass.AP(P21.tensor, P21.offset + 6 * XF + X0,
                    [[pst, NP], [7 * XF, 3], [1, 224]]),
        op=Alu.add)

    # depth weights
    if STAGE < 3:
        return
    for d in (-1, 1):
        v.memset(b.ds[d][:, :], 0.0)
        _emit_shift(nc, tcs, t, b.ds[d], "dstg", d, 1)
    dmap = [0, 1, 2, 3, 5, 6, 7, 8]
    for i, p in enumerate(dmap):
        dy, dx = _delta(p)
        src = b.dstg if dy == 0 else b.ds[dy]
        v.tensor_tensor(out=b.ddif[:, i, X0:X1],
                        in0=src[:, X0 + dx:X1 + dx],
                        in1=b.dstg[:, X0:X1], op=Alu.subtract)
    s.activation(out=b.dsq[:, :, X0:X1], in_=b.ddif[:, :, X0:X1], func=Act.Square)
    for di, wi, cls in ((0, 0, 0), (5, 6, 0), (1, 1, 1), (4, 5, 1)):
        s.activation(out=b.wb[:, wi:wi + 3:2, X0:X1],
                     in_=b.dsq[:, di:di + 3:2, X0:X1],
                     func=Act.Exp, scale=-50.0,
                     bias=b.bias2[:, cls:cls + 1])
    v.memset(b.wb[:, 4, X0:X1], 1.0)

    # wsum
    v.tensor_tensor(out=b.wsb[:, :, X0:X1], in0=b.wb[:, 0:3, X0:X1],
                    in1=b.wb[:, 3:6, X0:X1], op=Alu.add)
    v.tensor_tensor(out=b.wsb[:, :, X0:X1], in0=b.wsb[:, :, X0:X1],
                    in1=b.wb[:, 6:9, X0:X1], op=Alu.add)


def _emit_tile_post(nc, tcs, t, out):
    """Stage 3: cross-tile shifted versions of derived images + reductions."""
    b = tcs[t]
    v = nc.vector
    if STAGE < 4:
        nc.sync.dma_start(out=out[t], in_=b.acc[:, :])
        return

    for d in (-1, 1):
        v.memset(b.es[d][:, :], 0.0)
        _emit_shift(nc, tcs, t, b.es[d], "eimg", d, 1)
        v.memset(b.pis[d][:, :, :], 0.0)
        _emit_shift(nc, tcs, t, b.pis[d], "Pi", d, 3)
        v.memset(b.wss[d][:, :, :], 0.0)
        _emit_shift(nc, tcs, t, b.wss[d], "wsb", d, 3)

    # term1 + term2, batched per dy-group: the 3 p's of a group share dy and
    # read x-offsets -1,0,+1 -> one window AP (img-dim step 1 elem)
    for g in range(3):
        dy = g - 1
        e_src = b.eimg if dy == 0 else b.es[dy]
        est = e_src.ap[0][0]
        e_win = bass.AP(e_src.tensor, e_src.offset + (X0 - 1),
                        [[est, NP], [1, 3], [1, 224]])
        v.affine_mul_reduce(
            out=b.scr[:, :, X0:X1],
            accum_out=b.acc[:, g:g + 1],
            in0=b.wb[:, 3 * g:3 * g + 3, X0:X1],
            in1=e_win,
            scale=1.0, bias=0.0)
        pi_src = b.Pi if dy == 0 else b.pis[dy]
        pst = pi_src.ap[0][0]
        pi_win = bass.AP(pi_src.tensor, pi_src.offset + g * XF + (X0 - 1),
                         [[pst, NP], [1, 3], [1, 224]])
        v.affine_mul_reduce(
            out=b.scr[:, :, X0:X1],
            accum_out=b.acc[:, 9 + g:10 + g],
            in0=b.wb[:, 3 * g:3 * g + 3, X0:X1],
            in1=pi_win,
            scale=-2.0, bias=0.0)

    # term3
    def _T(q):
        dy, dx = _delta(q)
        src = b.wsb if dy == 0 else b.wss[-dy]
        return src[:, q % 3, X0 - dx:X1 - dx]

    for blk in range(3):
        v.tensor_tensor(out=b.omt[:, blk, X0:X1], in0=_T(3 * blk),
                        in1=_T(3 * blk + 1), op=Alu.add)
        v.tensor_tensor(out=b.omt[:, blk, X0:X1], in0=b.omt[:, blk, X0:X1],
                        in1=_T(3 * blk + 2), op=Alu.add)
    v.tensor_tensor(out=b.om[:, 0, X0:X1], in0=b.omt[:, 1, X0:X1],
                    in1=b.omt[:, 2, X0:X1], op=Alu.add)
    v.tensor_tensor(out=b.om[:, 1, X0:X1], in0=b.om[:, 0, X0:X1],
                    in1=b.omt[:, 0, X0:X1], op=Alu.add)
    v.tensor_tensor(out=b.om[:, 2, X0:X1], in0=b.omt[:, 0, X0:X1],
                    in1=b.omt[:, 1, X0:X1], op=Alu.add)
    v.affine_mul_reduce(
        out=b.scr[:, :, X0:X1],
        accum_out=b.acc[:, 18:19],
        in0=b.gsq[:, 9:12, X0:X1],
        in1=b.om[:, :, X0:X1],
        scale=3.0, bias=0.0)

    nc.sync.dma_start(out=out[t], in_=b.acc[:, :])


def build_nc():
    nc = bacc.Bacc("TRN2", target_bir_lowering=False)
    cam = nc.dram_tensor("cam", (C, H, W), F32, kind="ExternalInput")
    dep = nc.dram_tensor("dep", (1, H, W), F32, kind="ExternalInput")
    out = nc.dram_tensor("out", (2, NP, NACC), F32, kind="ExternalOutput")
    with tile.TileContext(nc) as tc:
        with tc.tile_pool(name="main", bufs=1) as pool:
            tcs = {t: _TileCtx(pool, t) for t in (0, 1)}
            for t in (0, 1):
                _emit_tile_pre(nc, tcs, t, cam, dep)
            for t in (0, 1):
                _emit_tile_main(nc, tcs, t, out)
            for t in (0, 1):
                _emit_tile_post(nc, tcs, t, out)
    nc.finalize()
    return nc


_CACHE = {}


def _get_nc():
    if "nc" not in _CACHE:
        _CACHE["nc"] = build_nc()
    return _CACHE["nc"]


def _run(in_maps, **kw):
    return run_bass_kernel_spmd(_get_nc(), in_maps, core_ids=list(range(N)), **kw)


def _make_in_maps(cam_map, depth_map):
    cam_map = np.ascontiguousarray(cam_map, dtype=np.float32)
    depth_map = np.ascontiguousarray(depth_map, dtype=np.float32)
    return [{"cam": cam_map[i], "dep": depth_map[i]} for i in range(N)]


def kernel(cam_map, depth_map):
    r = _run(_make_in_maps(cam_map, depth_map))
    tot = sum(float(m["out"].astype(np.float64).sum()) for m in r.results)
    return np.array(tot / (N * H * W), dtype=np.float32)



# revision 6
# speedup vs baseline: 1.9221x; 1.9221x over previous
"""DepthConsistencyLoss Trainium2 kernel v2 (8 NeuronCores, batch-parallel).

loss = mean_{n,l} sum_{r=0..188} w_{r%9}[l] * (cam_unfold[r,l] - cam_center[r%21,l])^2

Restructure (loss*N*H*W = sum_n T1 - 2*T2 + 3*T3'):
  Key identity: S_{-dp} w_p = w_{8-p} (spatial weights symmetric), so with
  masked weights w~_q = w_q * [l + dq inside image]:
    T1 = sum_l E * W~tot            E = sum_c cam_c^2, W~tot = sum_q w~_q
    T2 = sum_g sum_l R~_{2-g} * Pi_g   R~_h = row sums of w~,
         Pi_g = sum_{c0} P_c0 * S_{(dy,0)} cam_{c'}  (13 distinct products)
    T3' = sum_{c'} G_{c'} * Om_{c'}    Om from vertical band-matmuls of
         X = S_(0,1)wsum_0 + wsum_1 + S_(0,-1)wsum_2
  Masking is free: depth is loaded with BIG=1e4 padding, so invalid-shift
  weights come out exp(-50*BIG^2) = 0.

Layout: 2 tiles of 114 partitions = image rows; tile0 rows k=0..113,
tile1 REVERSED rows 223-k (so per-tile outputs m=0..111 start at partition 0
on both tiles). y-shifts are done on the idle PE with [114,112] shift/band
matrices (host-supplied constants); x-shifts are free-dim offsets.
Each core does one batch element; host sums the 8 x [2,112,8] partials.
"""

import os
import sys

import numpy as np

for _p in ("/opt/trn_rl_repo", os.path.expanduser("~/.axon_site/_ro/trn_rl_repo")):
    if os.path.isdir(_p) and _p not in sys.path:
        sys.path.insert(0, _p)

import concourse.bass as bass
import concourse.bacc as bacc
import concourse.tile as tile
from concourse import mybir
from concourse.bass_utils import run_bass_kernel_spmd

F32 = mybir.dt.float32
BF16 = mybir.dt.bfloat16
Alu = mybir.AluOpType
Act = mybir.ActivationFunctionType

N, C, H, W = 8, 21, 224, 224
KP = 114          # k-space partitions per tile (rows + 2 halo for PE shifts)
MP = 112          # m-space output rows per tile
XF = 228          # padded depth row: [2 pad][224][2 pad]
X0, X1 = 2, 226
NACC = 8
BIG = 1.0e4
WSPAT_D2 = [(p // 3 - 1) ** 2 + (p % 3 - 1) ** 2 for p in range(9)]

# 13 distinct products (dy, c0, c'), dy-major (see _tables in the reference
# restructure): runs of consecutive c0/c' per dy.
PRODS = [(-2, 0, 10), (-2, 1, 11),
         (-1, 4, 9), (-1, 5, 10), (-1, 6, 11),
         (0, 2, 9), (0, 3, 10), (0, 4, 11),
         (1, 0, 9), (1, 1, 10), (1, 2, 11),
         (2, 5, 9), (2, 6, 10)]
# group -> list of product indices (Pi_g = sum of those products)
GROUPS = {0: [8, 9, 10, 6, 7, 11, 12],
          1: [8, 9, 5, 6, 7, 3, 4],
          2: [0, 1, 5, 6, 2, 3, 4]}
# scam slots: products with dy != 0 need a materialized shifted cam image
SCAM = [(dy, cp) for (dy, c0, cp) in PRODS if dy != 0]   # 10 images
SCAM_IDX = {s: i for i, s in enumerate(SCAM)}

# wm matrix slots
MAT_B = {dy: 2 + dy for dy in (-2, -1, 0, 1, 2)}   # B_j at slot 2+j
MAT_DM, MAT_DT, MAT_DP = 5, 6, 7                   # B0+B-1, tri, B0+B+1
NMAT = 8


def _build_wm():
    """[NMAT, KP, MP] bf16 shift/band matrices W[k, m]."""
    wm = np.zeros((NMAT, KP, MP), np.float32)
    for j in (-2, -1, 0, 1, 2):
        for m in range(MP):
            k = m + j
            if 0 <= k < KP:
                wm[MAT_B[j], k, m] = 1.0
    wm[MAT_DM] = wm[MAT_B[0]] + wm[MAT_B[-1]]
    wm[MAT_DT] = wm[MAT_B[-1]] + wm[MAT_B[0]] + wm[MAT_B[1]]
    wm[MAT_DP] = wm[MAT_B[0]] + wm[MAT_B[1]]
    return wm


class _T:
    """Per-tile SBUF buffers."""

    def __init__(self, pool, t):
        self.t = t
        self.camb = pool.tile([KP, C, W], BF16, name=f"camb{t}", tag=f"camb{t}")
        self.dsh = pool.tile([KP, 3, XF], F32, name=f"dsh{t}", tag=f"dsh{t}")
        self.gsq = pool.tile([MP, C, W], BF16, name=f"gsq{t}", tag=f"gsq{t}")
        self.ddif = pool.tile([KP, 9, XF], BF16, name=f"ddif{t}", tag=f"ddif{t}")
        self.dsq = pool.tile([KP, 9, XF], BF16, name=f"dsq{t}", tag=f"dsq{t}")
        self.wb = pool.tile([KP, 9, XF], BF16, name=f"wb{t}", tag=f"wb{t}")
        self.wsum = pool.tile([KP, 3, XF], BF16, name=f"wsum{t}", tag=f"wsum{t}")
        self.xb = pool.tile([KP, XF], BF16, name=f"xb{t}", tag=f"xb{t}")
        self.wtot = pool.tile([MP, XF], BF16, name=f"wtot{t}", tag=f"wtot{t}")
        self.rb = pool.tile([MP, 3, XF], BF16, name=f"rb{t}", tag=f"rb{t}")
        self.pb = pool.tile([MP, 7, W], BF16, name=f"pb{t}", tag=f"pb{t}")
        self.scamb = pool.tile([MP, 10, W], BF16, name=f"scamb{t}", tag=f"scamb{t}")
        self.prod = pool.tile([MP, 13, W], BF16, name=f"prod{t}", tag=f"prod{t}")
        self.scr = pool.tile([MP, 3, W], BF16, name=f"scr{t}", tag=f"scr{t}")
        self.acc = pool.tile([MP, NACC], F32, name=f"acc{t}", tag=f"acc{t}")
        self.bias2 = pool.tile([KP, 2], F32, name=f"bias{t}", tag=f"bias{t}")


class _PS:
    """Shared PSUM tiles, one bank each (stride-256 keeps matmul outs in-bank);
    reused by both tiles (framework inserts WAR syncs)."""

    def __init__(self, ppool):
        self.e = ppool.tile([MP, 256], F32, name="e", tag="e")
        self.pi01 = ppool.tile([MP, 2, 256], F32, name="pi01", tag="pi01")
        self.pi2 = ppool.tile([MP, 256], F32, name="pi2", tag="pi2")
        self.om01 = ppool.tile([MP, 2, 256], F32, name="om01", tag="om01")
        self.om2 = ppool.tile([MP, 256], F32, name="om2", tag="om2")
        self.sc = ppool.tile([MP, 2, 256], F32, name="sc", tag="sc")
        self.sc2 = ppool.tile([MP, 2, 256], F32, name="sc2", tag="sc2")

    def pi(self, g):
        return self.pi01[:, g, 0:W] if g < 2 else self.pi2[:, 0:W]

    def om(self, g):
        return self.om01[:, g, 0:W] if g < 2 else self.om2[:, 0:W]


def _dram_ap(tensor, offset, dims):
    return bass.AP(tensor, offset, dims)


def _emit_loads(nc, b, t, cam, dep):
    """DMA loads for tile t. cam/dep are host-pre-reversed for t=1, so in
    both tiles buffer row k maps to source row k (tile1 source row k is
    image row 223-k) and shifts by d in row(k)-space are source rows k - d
    for t=1 (d flips sign in reversed space: handled by the caller passing
    the per-tile d ordering)."""
    sgn = 1 if t == 0 else -1

    # camb[k, c, x] = camsrc[c, k, x]
    nc.sync.dma_start(
        out=b.camb[:, :, :],
        in_=bass.AP(cam, 0, [[W, KP], [H * W, C], [1, W]]))

    # dsh slot di in (0,1,2) holds D(row(k) + (di-1)) = depsrc row k + sgn*(di-1)
    nc.vector.memset(b.dsh[:, :, :], BIG)
    for di in range(3):
        dd = sgn * (di - 1)                # source-row offset
        k0, k1 = max(0, -dd), min(KP, H - dd)   # valid k range
        nc.sync.dma_start(
            out=b.dsh[k0:k1, di, X0:X1],
            in_=bass.AP(dep, (k0 + dd) * W, [[W, k1 - k0], [1, W]]))


def _emit_wside(nc, b, t):
    """ddif -> dsq -> w~ -> wsum/R/Wtot/X."""
    v = nc.vector
    s = nc.scalar

    v.memset(b.bias2[:, 0:1], -0.02 * 1.0)
    v.memset(b.bias2[:, 1:2], -0.02 * 2.0)

    # ddif rows dy=-1,0,+1: window AP over 3 x-offsets vs broadcast center.
    # ddif[k, 3*di + i, x] = dsh[k, di, x + i - 1] - dsh[k, di(center)=1, x]
    # computed for x in [1, 227).
    dst, _ = b.dsh.ap[0][0], None
    for di in range(3):
        in0 = bass.AP(b.dsh.tensor, b.dsh.offset + di * XF + 0,
                      [[dst, KP], [1, 3], [1, 226]])
        in1 = bass.AP(b.dsh.tensor, b.dsh.offset + 1 * XF + 1,
                      [[dst, KP], [0, 3], [1, 226]])
        v.tensor_tensor(out=b.ddif[:, 3 * di:3 * di + 3, 1:227],
                        in0=in0, in1=in1, op=Alu.subtract)

    # dsq = ddif^2 (bf16 2x on DVE)
    v.tensor_tensor(out=b.dsq[:, :, 1:227], in0=b.ddif[:, :, 1:227],
                    in1=b.ddif[:, :, 1:227], op=Alu.mult)

    # w~ = exp(-50*dsq + ln wspat)  (3 instrs by wspat class)
    s.activation(out=b.wb[:, 1:8:2, 1:227], in_=b.dsq[:, 1:8:2, 1:227],
                 func=Act.Exp, scale=-50.0, bias=b.bias2[:, 0:1])
    s.activation(out=b.wb[:, 0:3:2, 1:227], in_=b.dsq[:, 0:3:2, 1:227],
                 func=Act.Exp, scale=-50.0, bias=b.bias2[:, 1:2])
    s.activation(out=b.wb[:, 6:9:2, 1:227], in_=b.dsq[:, 6:9:2, 1:227],
                 func=Act.Exp, scale=-50.0, bias=b.bias2[:, 1:2])
    s.activation(out=b.wb[:, 4, 1:227], in_=b.dsq[:, 4, 1:227],
                 func=Act.Exp, scale=-50.0)

    # wsum_m = w_m + w_{m+3} + w_{m+6}
    v.tensor_tensor(out=b.wsum[:, :, 1:227], in0=b.wb[:, 0:3, 1:227],
                    in1=b.wb[:, 3:6, 1:227], op=Alu.add)
    v.tensor_tensor(out=b.wsum[:, :, 1:227], in0=b.wsum[:, :, 1:227],
                    in1=b.wb[:, 6:9, 1:227], op=Alu.add)
    # R_h = w_{3h} + w_{3h+1} + w_{3h+2}  (img-stride-3 APs), m-space only
    wst = b.wb.ap[0][0]
    w_s3 = lambda q0, xoff: bass.AP(b.wb.tensor, b.wb.offset + q0 * XF + xoff,
                                    [[wst, MP], [3 * XF, 3], [1, 224]])
    v.tensor_tensor(out=b.rb[:, :, X0:X1], in0=w_s3(0, X0), in1=w_s3(1, X0),
                    op=Alu.add)
    v.tensor_tensor(out=b.rb[:, :, X0:X1], in0=b.rb[:, :, X0:X1],
                    in1=w_s3(2, X0), op=Alu.add)
    # Wtot = wsum_0 + wsum_1 + wsum_2
    v.tensor_tensor(out=b.wtot[:, X0:X1], in0=b.wsum[0:MP, 0, X0:X1],
                    in1=b.wsum[0:MP, 1, X0:X1], op=Alu.add)
    v.tensor_tensor(out=b.wtot[:, X0:X1], in0=b.wtot[:, X0:X1],
                    in1=b.wsum[0:MP, 2, X0:X1], op=Alu.add)
    # X = wsum_0(x+1) + wsum_1(x) + wsum_2(x-1), needs k up to 112
    v.tensor_tensor(out=b.xb[:, X0:X1], in0=b.wsum[:, 0, X0 + 1:X1 + 1],
                    in1=b.wsum[:, 1, X0:X1], op=Alu.add)
    v.tensor_tensor(out=b.xb[:, X0:X1], in0=b.xb[:, X0:X1],
                    in1=b.wsum[:, 2, X0 - 1:X1 - 1], op=Alu.add)


def _emit_camside(nc, b, ps, t, wmb):
    """squares, P, scam (PE), products, E/Pi (PE), Om (PE)."""
    v = nc.vector
    s = nc.scalar
    sgn = 1 if t == 0 else -1

    def mm(out, mat_slot, rhs, start, stop, kp=KP):
        nc.tensor.matmul(out=out, lhsT=wmb[0:kp, mat_slot, :], rhs=rhs,
                         start=start, stop=stop)

    # squares: ACT does 16 channels, DVE does 5
    s.activation(out=b.gsq[:, 0:16, :], in_=b.camb[0:MP, 0:16, :],
                 func=Act.Square)
    v.tensor_tensor(out=b.gsq[:, 16:21, :], in0=b.camb[0:MP, 16:21, :],
                    in1=b.camb[0:MP, 16:21, :], op=Alu.mult)

    # P_c0 = cam_c0 + cam_{c0+7} + cam_{c0+14}
    v.tensor_tensor(out=b.pb[:, :, :], in0=b.camb[0:MP, 0:7, :],
                    in1=b.camb[0:MP, 7:14, :], op=Alu.add)
    v.tensor_tensor(out=b.pb[:, :, :], in0=b.pb[:, :, :],
                    in1=b.camb[0:MP, 14:21, :], op=Alu.add)

    # scam via PE shift-matmuls (2 imgs per psum stage tile), ACT copies out
    for i0 in range(0, 10, 2):
        stage = ps.sc if (i0 // 2) % 2 == 0 else ps.sc2
        for j in range(2):
            dy, cp = SCAM[i0 + j]
            mm(stage[:, j, 0:W], MAT_B[sgn * dy], b.camb[:, cp, :],
               start=True, stop=True)
        s.activation(out=b.scamb[:, i0:i0 + 2, :],
                     in_=stage[:, :, 0:W], func=Act.Copy)

    # products, batched by dy-runs: (2, 3, 3, 3, 2) imgs
    runs = [(0, 2), (2, 3), (5, 3), (8, 3), (11, 2)]
    for r0, n in runs:
        dy, c0, cp = PRODS[r0]
        if dy == 0:
            src = b.camb[0:MP, cp:cp + n, :]
        else:
            s0 = SCAM_IDX[(dy, cp)]
            src = b.scamb[:, s0:s0 + n, :]
        v.tensor_tensor(out=b.prod[:, r0:r0 + n, :],
                        in0=b.pb[:, c0:c0 + n, :], in1=src, op=Alu.mult)

    # E = sum_c gsq_c (21 identity accum passes)
    for c in range(C):
        mm(ps.e[:, 0:W], MAT_B[0], b.gsq[:, c, :], start=(c == 0),
           stop=(c == C - 1), kp=MP)
    # Pi_g = sum over group products (7 identity accum passes each)
    for g in range(3):
        idxs = GROUPS[g]
        for i, pi_idx in enumerate(idxs):
            mm(ps.pi(g), MAT_B[0], b.prod[:, pi_idx, :], start=(i == 0),
               stop=(i == len(idxs) - 1), kp=MP)
    # Om: band matmuls on X  (tile1 swaps Om9/Om11 matrices)
    om_mats = (MAT_DM, MAT_DT, MAT_DP) if t == 0 else (MAT_DP, MAT_DT, MAT_DM)
    for g in range(3):
        mm(ps.om(g), om_mats[g], b.xb[:, X0:X1], start=True, stop=True)


def _emit_reduce(nc, b, ps, t, out):
    """amr reductions -> acc -> DRAM."""
    v = nc.vector
    rst = b.rb.ap[0][0]
    # T1: sum E * Wtot
    v.affine_mul_reduce(out=b.scr[:, 0, :], accum_out=b.acc[:, 0:1],
                        in0=b.wtot[:, X0:X1], in1=ps.e[:, 0:W],
                        scale=1.0, bias=0.0)
    # T2: sum_g R_{2-g} * Pi_g, scale -2   (R reversed via negative stride)
    r_rev01 = bass.AP(b.rb.tensor, b.rb.offset + 2 * XF + X0,
                      [[rst, MP], [-XF, 2], [1, 224]])
    v.affine_mul_reduce(out=b.scr[:, 0:2, :], accum_out=b.acc[:, 1:2],
                        in0=r_rev01, in1=ps.pi01[:, :, 0:W],
                        scale=-2.0, bias=0.0)
    v.affine_mul_reduce(out=b.scr[:, 2, :], accum_out=b.acc[:, 2:3],
                        in0=b.rb[:, 0, X0:X1], in1=ps.pi2[:, 0:W],
                        scale=-2.0, bias=0.0)
    # T3: 3 * sum_c' G_c' * Om_c'
    v.affine_mul_reduce(out=b.scr[:, 0:2, :], accum_out=b.acc[:, 3:4],
                        in0=b.gsq[:, 9:11, :], in1=ps.om01[:, :, 0:W],
                        scale=3.0, bias=0.0)
    v.affine_mul_reduce(out=b.scr[:, 2, :], accum_out=b.acc[:, 4:5],
                        in0=b.gsq[:, 11, :], in1=ps.om2[:, 0:W],
                        scale=3.0, bias=0.0)
    v.memset(b.acc[:, 5:8], 0.0)
    nc.sync.dma_start(out=out[t], in_=b.acc[:, :])


def build_nc():
    nc = bacc.Bacc("TRN2", target_bir_lowering=False)
    cam = nc.dram_tensor("cam", (C, H, W), BF16, kind="ExternalInput")
    dep = nc.dram_tensor("dep", (H, W), F32, kind="ExternalInput")
    camr = nc.dram_tensor("camr", (C, H, W), BF16, kind="ExternalInput")
    depr = nc.dram_tensor("depr", (H, W), F32, kind="ExternalInput")
    wm = nc.dram_tensor("wm", (NMAT, KP, MP), BF16, kind="ExternalInput")
    out = nc.dram_tensor("out", (2, MP, NACC), F32, kind="ExternalOutput")
    with tile.TileContext(nc) as tc:
        with tc.tile_pool(name="main", bufs=1) as pool, \
             tc.tile_pool(name="psum", bufs=1, space="PSUM") as ppool:
            wmb = pool.tile([KP, NMAT, MP], BF16, name="wmb", tag="wmb")
            nc.sync.dma_start(
                out=wmb[:, :, :],
                in_=bass.AP(wm, 0, [[MP, KP], [KP * MP, NMAT], [1, MP]]))
            bs = {t: _T(pool, t) for t in (0, 1)}
            ps = _PS(ppool)
            _emit_loads(nc, bs[0], 0, cam, dep)
            _emit_loads(nc, bs[1], 1, camr, depr)
            for t in (0, 1):
                _emit_wside(nc, bs[t], t)
                _emit_camside(nc, bs[t], ps, t, wmb)
                _emit_reduce(nc, bs[t], ps, t, out)
    nc.finalize()
    return nc


_CACHE = {}


def _get_nc():
    if "nc" not in _CACHE:
        _CACHE["nc"] = build_nc()
    return _CACHE["nc"]


def _run(in_maps, **kw):
    return run_bass_kernel_spmd(_get_nc(), in_maps, core_ids=list(range(N)), **kw)


def _make_in_maps(cam_map, depth_map):
    import ml_dtypes
    camb = np.ascontiguousarray(cam_map, dtype=np.float32).astype(ml_dtypes.bfloat16)
    dep = np.ascontiguousarray(depth_map, dtype=np.float32)
    cambr = np.ascontiguousarray(camb[:, :, ::-1, :])
    depre = np.ascontiguousarray(dep[:, :, ::-1, :])
    wm = _build_wm().astype(ml_dtypes.bfloat16)
    return [{"cam": camb[i], "dep": dep[i, 0], "camr": cambr[i],
             "depr": depre[i, 0], "wm": wm} for i in range(N)]


def kernel(cam_map, depth_map):
    r = _run(_make_in_maps(cam_map, depth_map))
    tot = sum(float(m["out"].astype(np.float64).sum()) for m in r.results)
    return np.array(tot / (N * H * W), dtype=np.float32)


# revision 8
# speedup vs baseline: 2.0794x; 1.0818x over previous
"""DepthConsistencyLoss Trainium2 kernel v2 (8 NeuronCores, batch-parallel).

loss = mean_{n,l} sum_{r=0..188} w_{r%9}[l] * (cam_unfold[r,l] - cam_center[r%21,l])^2

Restructure (loss*N*H*W = sum_n T1 - 2*T2 + 3*T3'):
  Key identity: S_{-dp} w_p = w_{8-p} (spatial weights symmetric), so with
  masked weights w~_q = w_q * [l + dq inside image]:
    T1 = sum_l E * W~tot            E = sum_c cam_c^2, W~tot = sum_q w~_q
    T2 = sum_g sum_l R~_{2-g} * Pi_g   R~_h = row sums of w~,
         Pi_g = sum_{c0} P_c0 * S_{(dy,0)} cam_{c'}  (13 distinct products)
    T3' = sum_{c'} G_{c'} * Om_{c'}    Om from vertical band-matmuls of
         X = S_(0,1)wsum_0 + wsum_1 + S_(0,-1)wsum_2
  Masking is free: depth is loaded with BIG=1e4 padding, so invalid-shift
  weights come out exp(-50*BIG^2) = 0.

Layout: 2 tiles of 114 partitions = image rows; tile0 rows k=0..113,
tile1 REVERSED rows 223-k (so per-tile outputs m=0..111 start at partition 0
on both tiles). y-shifts are done on the idle PE with [114,112] shift/band
matrices (host-supplied constants); x-shifts are free-dim offsets.
Each core does one batch element; host sums the 8 x [2,112,8] partials.
"""

import os
import sys

import numpy as np

for _p in ("/opt/trn_rl_repo", os.path.expanduser("~/.axon_site/_ro/trn_rl_repo")):
    if os.path.isdir(_p) and _p not in sys.path:
        sys.path.insert(0, _p)

import concourse.bass as bass
import concourse.bacc as bacc
import concourse.tile as tile
from concourse import mybir
from concourse.bass_utils import run_bass_kernel_spmd

F32 = mybir.dt.float32
BF16 = mybir.dt.bfloat16
Alu = mybir.AluOpType
Act = mybir.ActivationFunctionType

N, C, H, W = 8, 21, 224, 224
KP = 114          # k-space partitions per tile (rows + 2 halo for PE shifts)
MP = 112          # m-space output rows per tile
XF = 228          # padded depth row: [2 pad][224][2 pad]
X0, X1 = 2, 226
NACC = 5
BIG = 1.0e4
WSPAT_D2 = [(p // 3 - 1) ** 2 + (p % 3 - 1) ** 2 for p in range(9)]

# 13 distinct products (dy, c0, c'), dy-major (see _tables in the reference
# restructure): runs of consecutive c0/c' per dy.
PRODS = [(-2, 0, 10), (-2, 1, 11),
         (-1, 4, 9), (-1, 5, 10), (-1, 6, 11),
         (0, 2, 9), (0, 3, 10), (0, 4, 11),
         (1, 0, 9), (1, 1, 10), (1, 2, 11),
         (2, 5, 9), (2, 6, 10)]
# group -> list of product indices (Pi_g = sum of those products)
GROUPS = {0: [8, 9, 10, 6, 7, 11, 12],
          1: [8, 9, 5, 6, 7, 3, 4],
          2: [0, 1, 5, 6, 2, 3, 4]}
# scam slots: products with dy != 0 need a materialized shifted cam image
SCAM = [(dy, cp) for (dy, c0, cp) in PRODS if dy != 0]   # 10 images
SCAM_IDX = {s: i for i, s in enumerate(SCAM)}

# wm matrix slots
MAT_B = {dy: 2 + dy for dy in (-2, -1, 0, 1, 2)}   # B_j at slot 2+j
MAT_DM, MAT_DT, MAT_DP = 5, 6, 7                   # B0+B-1, tri, B0+B+1
NMAT = 8


def _build_wm():
    """[NMAT, KP, MP] bf16 shift/band matrices W[k, m]."""
    wm = np.zeros((NMAT, KP, MP), np.float32)
    for j in (-2, -1, 0, 1, 2):
        for m in range(MP):
            k = m + j
            if 0 <= k < KP:
                wm[MAT_B[j], k, m] = 1.0
    wm[MAT_DM] = wm[MAT_B[0]] + wm[MAT_B[-1]]
    wm[MAT_DT] = wm[MAT_B[-1]] + wm[MAT_B[0]] + wm[MAT_B[1]]
    wm[MAT_DP] = wm[MAT_B[0]] + wm[MAT_B[1]]
    return wm


class _T:
    """Per-tile SBUF buffers."""

    def __init__(self, pool, t):
        self.t = t
        self.camb = pool.tile([KP, C, W], BF16, name=f"camb{t}", tag=f"camb{t}")
        self.dsh = pool.tile([KP, 3, XF], BF16, name=f"dsh{t}", tag=f"dsh{t}")
        self.gsq = pool.tile([MP, C, W], BF16, name=f"gsq{t}", tag=f"gsq{t}")
        self.ddif = pool.tile([KP, 9, XF], BF16, name=f"ddif{t}", tag=f"ddif{t}")
        self.dsq = pool.tile([KP, 9, XF], BF16, name=f"dsq{t}", tag=f"dsq{t}")
        self.wb = pool.tile([KP, 9, XF], BF16, name=f"wb{t}", tag=f"wb{t}")
        self.wsum = pool.tile([KP, 3, XF], BF16, name=f"wsum{t}", tag=f"wsum{t}")
        self.xb = pool.tile([KP, XF], BF16, name=f"xb{t}", tag=f"xb{t}")
        self.wtot = pool.tile([MP, XF], BF16, name=f"wtot{t}", tag=f"wtot{t}")
        self.rb = pool.tile([MP, 3, XF], BF16, name=f"rb{t}", tag=f"rb{t}")
        self.pb = pool.tile([MP, 7, W], BF16, name=f"pb{t}", tag=f"pb{t}")
        self.scamb = pool.tile([MP, 10, W], BF16, name=f"scamb{t}", tag=f"scamb{t}")
        self.prod = pool.tile([MP, 13, W], BF16, name=f"prod{t}", tag=f"prod{t}")
        self.scr = pool.tile([MP, 3, W], BF16, name=f"scr{t}", tag=f"scr{t}")
        self.acc = pool.tile([MP, NACC], F32, name=f"acc{t}", tag=f"acc{t}")
        self.bias2 = pool.tile([KP, 2], F32, name=f"bias{t}", tag=f"bias{t}")


class _PS:
    """Shared PSUM tiles, one bank each (stride-256 keeps matmul outs in-bank);
    reused by both tiles (framework inserts WAR syncs)."""

    def __init__(self, ppool):
        self.e = ppool.tile([MP, 256], F32, name="e", tag="e")
        self.pi01 = ppool.tile([MP, 2, 256], F32, name="pi01", tag="pi01")
        self.pi2 = ppool.tile([MP, 256], F32, name="pi2", tag="pi2")
        self.om01 = ppool.tile([MP, 2, 256], F32, name="om01", tag="om01")
        self.om2 = ppool.tile([MP, 256], F32, name="om2", tag="om2")
        self.sc = ppool.tile([MP, 2, 256], F32, name="sc", tag="sc")
        self.sc2 = ppool.tile([MP, 2, 256], F32, name="sc2", tag="sc2")
        self.sc3 = ppool.tile([MP, 2, 256], F32, name="sc3", tag="sc3")

    def pi(self, g):
        return self.pi01[:, g, 0:W] if g < 2 else self.pi2[:, 0:W]

    def om(self, g):
        return self.om01[:, g, 0:W] if g < 2 else self.om2[:, 0:W]


def _dram_ap(tensor, offset, dims):
    return bass.AP(tensor, offset, dims)


def _emit_loads(nc, b, t, cam, dep):
    """DMA loads for tile t. cam/dep are host-pre-reversed for t=1, so in
    both tiles buffer row k maps to source row k (tile1 source row k is
    image row 223-k) and shifts by d in row(k)-space are source rows k - d
    for t=1 (d flips sign in reversed space: handled by the caller passing
    the per-tile d ordering)."""
    sgn = 1 if t == 0 else -1

    # dsh slot di in (0,1,2) holds D(row(k) + (di-1)) = depsrc row k + sgn*(di-1)
    nc.gpsimd.memset(b.dsh[:, :, :], BIG)
    for di in range(3):
        dd = sgn * (di - 1)                # source-row offset
        k0, k1 = max(0, -dd), min(KP, H - dd)   # valid k range
        nc.sync.dma_start(
            out=b.dsh[k0:k1, di, X0:X1],
            in_=bass.AP(dep, (k0 + dd) * W, [[W, k1 - k0], [1, W]]))


CHUNKS = [(0, 8), (8, 16), (16, 21)]


def _emit_cam_chunk(nc, b, cam, c0, c1):
    nc.sync.dma_start(
        out=b.camb[:, c0:c1, :],
        in_=bass.AP(cam, c0 * H * W, [[W, KP], [H * W, c1 - c0], [1, W]]))


def _emit_wside(nc, b, t):
    """ddif -> dsq -> w~ -> wsum/R/Wtot/X."""
    v = nc.vector
    s = nc.scalar

    nc.gpsimd.memset(b.bias2[:, 0:1], -0.02 * 1.0)
    nc.gpsimd.memset(b.bias2[:, 1:2], -0.02 * 2.0)

    # ddif rows dy=-1,0,+1: window AP over 3 x-offsets vs broadcast center.
    # ddif[k, 3*di + i, x] = dsh[k, di, x + i - 1] - dsh[k, di(center)=1, x]
    # computed for x in [1, 227).
    dst, _ = b.dsh.ap[0][0], None
    for di in range(3):
        in0 = bass.AP(b.dsh.tensor, b.dsh.offset + di * XF + 0,
                      [[dst, KP], [1, 3], [1, 226]])
        in1 = bass.AP(b.dsh.tensor, b.dsh.offset + 1 * XF + 1,
                      [[dst, KP], [0, 3], [1, 226]])
        v.tensor_tensor(out=b.ddif[:, 3 * di:3 * di + 3, 1:227],
                        in0=in0, in1=in1, op=Alu.subtract)

    # dsq = ddif^2 (bf16 2x on DVE)
    v.tensor_tensor(out=b.dsq[:, :, 1:227], in0=b.ddif[:, :, 1:227],
                    in1=b.ddif[:, :, 1:227], op=Alu.mult)

    # w~ = exp(-50*dsq + ln wspat)  (3 instrs by wspat class)
    s.activation(out=b.wb[:, 1:8:2, 1:227], in_=b.dsq[:, 1:8:2, 1:227],
                 func=Act.Exp, scale=-50.0, bias=b.bias2[:, 0:1])
    s.activation(out=b.wb[:, 0:3:2, 1:227], in_=b.dsq[:, 0:3:2, 1:227],
                 func=Act.Exp, scale=-50.0, bias=b.bias2[:, 1:2])
    s.activation(out=b.wb[:, 6:9:2, 1:227], in_=b.dsq[:, 6:9:2, 1:227],
                 func=Act.Exp, scale=-50.0, bias=b.bias2[:, 1:2])
    s.activation(out=b.wb[:, 4, 1:227], in_=b.dsq[:, 4, 1:227],
                 func=Act.Exp, scale=-50.0)

    # wsum_m = w_m + w_{m+3} + w_{m+6}
    v.tensor_tensor(out=b.wsum[:, :, 1:227], in0=b.wb[:, 0:3, 1:227],
                    in1=b.wb[:, 3:6, 1:227], op=Alu.add)
    v.tensor_tensor(out=b.wsum[:, :, 1:227], in0=b.wsum[:, :, 1:227],
                    in1=b.wb[:, 6:9, 1:227], op=Alu.add)
    # R_h = w_{3h} + w_{3h+1} + w_{3h+2}  (img-stride-3 APs), m-space only
    wst = b.wb.ap[0][0]
    w_s3 = lambda q0, xoff: bass.AP(b.wb.tensor, b.wb.offset + q0 * XF + xoff,
                                    [[wst, MP], [3 * XF, 3], [1, 224]])
    v.tensor_tensor(out=b.rb[:, :, X0:X1], in0=w_s3(0, X0), in1=w_s3(1, X0),
                    op=Alu.add)
    v.tensor_tensor(out=b.rb[:, :, X0:X1], in0=b.rb[:, :, X0:X1],
                    in1=w_s3(2, X0), op=Alu.add)
    # Wtot = wsum_0 + wsum_1 + wsum_2
    v.tensor_tensor(out=b.wtot[:, X0:X1], in0=b.wsum[0:MP, 0, X0:X1],
                    in1=b.wsum[0:MP, 1, X0:X1], op=Alu.add)
    v.tensor_tensor(out=b.wtot[:, X0:X1], in0=b.wtot[:, X0:X1],
                    in1=b.wsum[0:MP, 2, X0:X1], op=Alu.add)
    # X = wsum_0(x+1) + wsum_1(x) + wsum_2(x-1), needs k up to 112
    v.tensor_tensor(out=b.xb[:, X0:X1], in0=b.wsum[:, 0, X0 + 1:X1 + 1],
                    in1=b.wsum[:, 1, X0:X1], op=Alu.add)
    v.tensor_tensor(out=b.xb[:, X0:X1], in0=b.xb[:, X0:X1],
                    in1=b.wsum[:, 2, X0 - 1:X1 - 1], op=Alu.add)


def _emit_camside(nc, b, ps, t, wmb):
    """squares, P, scam (PE), products, E/Pi (PE), Om (PE)."""
    v = nc.vector
    s = nc.scalar
    sgn = 1 if t == 0 else -1

    def mm(out, mat_slot, rhs, start, stop, kp=KP):
        nc.tensor.matmul(out=out, lhsT=wmb[0:kp, mat_slot, :], rhs=rhs,
                         start=start, stop=stop)

    # squares: ACT does 14 channels, DVE does 7 (per-chunk granularity)
    s.activation(out=b.gsq[:, 0:8, :], in_=b.camb[0:MP, 0:8, :],
                 func=Act.Square)
    s.activation(out=b.gsq[:, 8:14, :], in_=b.camb[0:MP, 8:14, :],
                 func=Act.Square)
    v.tensor_tensor(out=b.gsq[:, 14:16, :], in0=b.camb[0:MP, 14:16, :],
                    in1=b.camb[0:MP, 14:16, :], op=Alu.mult)
    v.tensor_tensor(out=b.gsq[:, 16:21, :], in0=b.camb[0:MP, 16:21, :],
                    in1=b.camb[0:MP, 16:21, :], op=Alu.mult)

    # P_c0 = cam_c0 + cam_{c0+7} + cam_{c0+14}
    v.tensor_tensor(out=b.pb[:, :, :], in0=b.camb[0:MP, 0:7, :],
                    in1=b.camb[0:MP, 7:14, :], op=Alu.add)
    v.tensor_tensor(out=b.pb[:, :, :], in0=b.pb[:, :, :],
                    in1=b.camb[0:MP, 14:21, :], op=Alu.add)

    # scam via PE shift-matmuls (2 imgs per psum stage tile); copies out on
    # ACT (3 instrs) and Pool (2 instrs)
    stages = [ps.sc, ps.sc2, ps.sc3]
    for si, i0 in enumerate(range(0, 10, 2)):
        stage = stages[si % 3]
        for j in range(2):
            dy, cp = SCAM[i0 + j]
            mm(stage[:, j, 0:W], MAT_B[sgn * dy], b.camb[:, cp, :],
               start=True, stop=True)
        if si < 3:
            s.activation(out=b.scamb[:, i0:i0 + 2, :],
                         in_=stage[:, :, 0:W], func=Act.Copy)
        else:
            v.tensor_copy(out=b.scamb[:, i0:i0 + 2, :],
                          in_=stage[:, :, 0:W])

    # products, batched by dy-runs: (2, 3, 3, 3, 2) imgs
    runs = [(0, 2), (2, 3), (5, 3), (8, 3), (11, 2)]
    for r0, n in runs:
        dy, c0, cp = PRODS[r0]
        if dy == 0:
            src = b.camb[0:MP, cp:cp + n, :]
        else:
            s0 = SCAM_IDX[(dy, cp)]
            src = b.scamb[:, s0:s0 + n, :]
        v.tensor_tensor(out=b.prod[:, r0:r0 + n, :],
                        in0=b.pb[:, c0:c0 + n, :], in1=src, op=Alu.mult)

    # E = sum_c gsq_c (21 identity accum passes)
    for c in range(C):
        mm(ps.e[:, 0:W], MAT_B[0], b.gsq[:, c, :], start=(c == 0),
           stop=(c == C - 1), kp=MP)
    # Pi_g = sum over group products (7 identity accum passes each)
    for g in range(3):
        idxs = GROUPS[g]
        for i, pi_idx in enumerate(idxs):
            mm(ps.pi(g), MAT_B[0], b.prod[:, pi_idx, :], start=(i == 0),
               stop=(i == len(idxs) - 1), kp=MP)
    # Om: band matmuls on X  (tile1 swaps Om9/Om11 matrices)
    om_mats = (MAT_DM, MAT_DT, MAT_DP) if t == 0 else (MAT_DP, MAT_DT, MAT_DM)
    for g in range(3):
        mm(ps.om(g), om_mats[g], b.xb[:, X0:X1], start=True, stop=True)


def _emit_reduce(nc, b, ps, t, out):
    """amr reductions -> acc -> DRAM."""
    v = nc.vector
    rst = b.rb.ap[0][0]
    # T1: sum E * Wtot
    v.affine_mul_reduce(out=b.scr[:, 0, :], accum_out=b.acc[:, 0:1],
                        in0=b.wtot[:, X0:X1], in1=ps.e[:, 0:W],
                        scale=1.0, bias=0.0)
    # T2: sum_g R_{2-g} * Pi_g, scale -2   (R reversed via negative stride)
    r_rev01 = bass.AP(b.rb.tensor, b.rb.offset + 2 * XF + X0,
                      [[rst, MP], [-XF, 2], [1, 224]])
    v.affine_mul_reduce(out=b.scr[:, 0:2, :], accum_out=b.acc[:, 1:2],
                        in0=r_rev01, in1=ps.pi01[:, :, 0:W],
                        scale=-2.0, bias=0.0)
    v.affine_mul_reduce(out=b.scr[:, 2, :], accum_out=b.acc[:, 2:3],
                        in0=b.rb[:, 0, X0:X1], in1=ps.pi2[:, 0:W],
                        scale=-2.0, bias=0.0)
    # T3: 3 * sum_c' G_c' * Om_c'
    v.affine_mul_reduce(out=b.scr[:, 0:2, :], accum_out=b.acc[:, 3:4],
                        in0=b.gsq[:, 9:11, :], in1=ps.om01[:, :, 0:W],
                        scale=3.0, bias=0.0)
    v.affine_mul_reduce(out=b.scr[:, 2, :], accum_out=b.acc[:, 4:5],
                        in0=b.gsq[:, 11, :], in1=ps.om2[:, 0:W],
                        scale=3.0, bias=0.0)
    nc.sync.dma_start(out=out[t], in_=b.acc[:, :])


def build_nc():
    nc = bacc.Bacc("TRN2", target_bir_lowering=False)
    cam = nc.dram_tensor("cam", (C, H, W), BF16, kind="ExternalInput")
    dep = nc.dram_tensor("dep", (H, W), BF16, kind="ExternalInput")
    camr = nc.dram_tensor("camr", (C, H, W), BF16, kind="ExternalInput")
    depr = nc.dram_tensor("depr", (H, W), BF16, kind="ExternalInput")
    wm = nc.dram_tensor("wm", (NMAT, KP, MP), BF16, kind="ExternalInput")
    out = nc.dram_tensor("out", (2, MP, NACC), F32, kind="ExternalOutput")
    with tile.TileContext(nc) as tc:
        with tc.tile_pool(name="main", bufs=1) as pool, \
             tc.tile_pool(name="psum", bufs=1, space="PSUM") as ppool:
            wmb = pool.tile([KP, NMAT, MP], BF16, name="wmb", tag="wmb")
            nc.sync.dma_start(
                out=wmb[:, :, :],
                in_=bass.AP(wm, 0, [[MP, KP], [KP * MP, NMAT], [1, MP]]))
            bs = {t: _T(pool, t) for t in (0, 1)}
            ps = _PS(ppool)
            _emit_loads(nc, bs[0], 0, cam, dep)
            _emit_loads(nc, bs[1], 1, camr, depr)
            for c0, c1 in CHUNKS:
                _emit_cam_chunk(nc, bs[0], cam, c0, c1)
                _emit_cam_chunk(nc, bs[1], camr, c0, c1)
            _emit_wside(nc, bs[0], 0)
            _emit_wside(nc, bs[1], 1)
            for t in (0, 1):
                _emit_camside(nc, bs[t], ps, t, wmb)
                _emit_reduce(nc, bs[t], ps, t, out)
    nc.finalize()
    return nc


_CACHE = {}


def _get_nc():
    if "nc" not in _CACHE:
        _CACHE["nc"] = build_nc()
    return _CACHE["nc"]


def _run(in_maps, **kw):
    return run_bass_kernel_spmd(_get_nc(), in_maps, core_ids=list(range(N)), **kw)


def _make_in_maps(cam_map, depth_map):
    import ml_dtypes
    camb = np.ascontiguousarray(cam_map, dtype=np.float32).astype(ml_dtypes.bfloat16)
    dep = np.ascontiguousarray(depth_map, dtype=np.float32).astype(ml_dtypes.bfloat16)
    cambr = np.ascontiguousarray(camb[:, :, ::-1, :])
    depre = np.ascontiguousarray(dep[:, :, ::-1, :])
    wm = _build_wm().astype(ml_dtypes.bfloat16)
    return [{"cam": camb[i], "dep": dep[i, 0], "camr": cambr[i],
             "depr": depre[i, 0], "wm": wm} for i in range(N)]


def kernel(cam_map, depth_map):
    r = _run(_make_in_maps(cam_map, depth_map))
    tot = sum(float(m["out"].astype(np.float64).sum()) for m in r.results)
    return np.array(tot / (N * H * W), dtype=np.float32)


# revision 11
# speedup vs baseline: 2.1273x; 1.0230x over previous
"""DepthConsistencyLoss Trainium2 kernel v2 (8 NeuronCores, batch-parallel).

loss = mean_{n,l} sum_{r=0..188} w_{r%9}[l] * (cam_unfold[r,l] - cam_center[r%21,l])^2

Restructure (loss*N*H*W = sum_n T1 - 2*T2 + 3*T3'):
  Key identity: S_{-dp} w_p = w_{8-p} (spatial weights symmetric), so with
  masked weights w~_q = w_q * [l + dq inside image]:
    T1 = sum_l E * W~tot            E = sum_c cam_c^2, W~tot = sum_q w~_q
    T2 = sum_g sum_l R~_{2-g} * Pi_g   R~_h = row sums of w~,
         Pi_g = sum_{c0} P_c0 * S_{(dy,0)} cam_{c'}  (13 distinct products)
    T3' = sum_{c'} G_{c'} * Om_{c'}    Om from vertical band-matmuls of
         X = S_(0,1)wsum_0 + wsum_1 + S_(0,-1)wsum_2
  Masking is free: depth is loaded with BIG=1e4 padding, so invalid-shift
  weights come out exp(-50*BIG^2) = 0.

Layout: 2 tiles of 114 partitions = image rows; tile0 rows k=0..113,
tile1 REVERSED rows 223-k (so per-tile outputs m=0..111 start at partition 0
on both tiles). y-shifts are done on the idle PE with [114,112] shift/band
matrices (host-supplied constants); x-shifts are free-dim offsets.
Each core does one batch element; host sums the 8 x [2,112,8] partials.
"""

import os
import sys

import numpy as np

for _p in ("/opt/trn_rl_repo", os.path.expanduser("~/.axon_site/_ro/trn_rl_repo")):
    if os.path.isdir(_p) and _p not in sys.path:
        sys.path.insert(0, _p)

import concourse.bass as bass
import concourse.bacc as bacc
import concourse.tile as tile
from concourse import mybir
from concourse.bass_utils import run_bass_kernel_spmd

F32 = mybir.dt.float32
BF16 = mybir.dt.bfloat16
Alu = mybir.AluOpType
Act = mybir.ActivationFunctionType

N, C, H, W = 8, 21, 224, 224
KP = 114          # k-space partitions per tile (rows + 2 halo for PE shifts)
MP = 112          # m-space output rows per tile
XF = 228          # padded depth row: [2 pad][224][2 pad]
X0, X1 = 2, 226
NACC = 5
BIG = 1.0e4
WSPAT_D2 = [(p // 3 - 1) ** 2 + (p % 3 - 1) ** 2 for p in range(9)]

# 13 distinct products (dy, c0, c'), dy-major (see _tables in the reference
# restructure): runs of consecutive c0/c' per dy.
PRODS = [(-2, 0, 10), (-2, 1, 11),
         (-1, 4, 9), (-1, 5, 10), (-1, 6, 11),
         (0, 2, 9), (0, 3, 10), (0, 4, 11),
         (1, 0, 9), (1, 1, 10), (1, 2, 11),
         (2, 5, 9), (2, 6, 10)]
# group -> list of product indices (Pi_g = sum of those products)
GROUPS = {0: [8, 9, 10, 6, 7, 11, 12],
          1: [8, 9, 5, 6, 7, 3, 4],
          2: [0, 1, 5, 6, 2, 3, 4]}
# scam slots: products with dy != 0 need a materialized shifted cam image
SCAM = [(dy, cp) for (dy, c0, cp) in PRODS if dy != 0]   # 10 images
SCAM_IDX = {s: i for i, s in enumerate(SCAM)}

# wm matrix slots
MAT_B = {dy: 2 + dy for dy in (-2, -1, 0, 1, 2)}   # B_j at slot 2+j
MAT_DM, MAT_DT, MAT_DP = 5, 6, 7                   # B0+B-1, tri, B0+B+1
NMAT = 8


def _build_wm():
    """[NMAT, KP, MP] bf16 shift/band matrices W[k, m]."""
    wm = np.zeros((NMAT, KP, MP), np.float32)
    for j in (-2, -1, 0, 1, 2):
        for m in range(MP):
            k = m + j
            if 0 <= k < KP:
                wm[MAT_B[j], k, m] = 1.0
    wm[MAT_DM] = wm[MAT_B[0]] + wm[MAT_B[-1]]
    wm[MAT_DT] = wm[MAT_B[-1]] + wm[MAT_B[0]] + wm[MAT_B[1]]
    wm[MAT_DP] = wm[MAT_B[0]] + wm[MAT_B[1]]
    return wm


class _T:
    """Per-tile SBUF buffers."""

    def __init__(self, pool, t):
        self.t = t
        self.camb = pool.tile([KP, C, W], BF16, name=f"camb{t}", tag=f"camb{t}")
        self.dsh = pool.tile([KP, 3, XF], BF16, name=f"dsh{t}", tag=f"dsh{t}")
        self.gsq = pool.tile([MP, C, W], BF16, name=f"gsq{t}", tag=f"gsq{t}")
        self.ddif = pool.tile([KP, 9, XF], BF16, name=f"ddif{t}", tag=f"ddif{t}")
        self.dsq = pool.tile([KP, 9, XF], BF16, name=f"dsq{t}", tag=f"dsq{t}")
        self.wb = pool.tile([KP, 9, XF], BF16, name=f"wb{t}", tag=f"wb{t}")
        self.wsum = pool.tile([KP, 3, XF], BF16, name=f"wsum{t}", tag=f"wsum{t}")
        self.xb = pool.tile([KP, XF], BF16, name=f"xb{t}", tag=f"xb{t}")
        self.wtot = pool.tile([MP, XF], BF16, name=f"wtot{t}", tag=f"wtot{t}")
        self.rb = pool.tile([MP, 3, XF], BF16, name=f"rb{t}", tag=f"rb{t}")
        self.pb = pool.tile([MP, 7, W], BF16, name=f"pb{t}", tag=f"pb{t}")
        self.scamb = pool.tile([MP, 10, W], BF16, name=f"scamb{t}", tag=f"scamb{t}")
        self.prod = pool.tile([MP, 13, W], BF16, name=f"prod{t}", tag=f"prod{t}")
        self.scr = pool.tile([MP, 3, W], BF16, name=f"scr{t}", tag=f"scr{t}")
        self.acc = pool.tile([MP, NACC], F32, name=f"acc{t}", tag=f"acc{t}")
        self.bias2 = pool.tile([KP, 2], F32, name=f"bias{t}", tag=f"bias{t}")


class _PS:
    """Shared PSUM tiles, one bank each (stride-256 keeps matmul outs in-bank);
    reused by both tiles (framework inserts WAR syncs)."""

    def __init__(self, ppool):
        self.e = {0: ppool.tile([MP, 256], F32, name="e0", tag="e0"),
                  1: ppool.tile([MP, 256], F32, name="e1", tag="e1")}
        self.pi01 = ppool.tile([MP, 2, 256], F32, name="pi01", tag="pi01")
        self.pi2 = ppool.tile([MP, 256], F32, name="pi2", tag="pi2")
        self.om01 = ppool.tile([MP, 2, 256], F32, name="om01", tag="om01")
        self.om2 = ppool.tile([MP, 256], F32, name="om2", tag="om2")
        self.sc = ppool.tile([MP, 2, 256], F32, name="sc", tag="sc")
        self.sc2 = ppool.tile([MP, 2, 256], F32, name="sc2", tag="sc2")

    def pi(self, g):
        return self.pi01[:, g, 0:W] if g < 2 else self.pi2[:, 0:W]

    def om(self, g):
        return self.om01[:, g, 0:W] if g < 2 else self.om2[:, 0:W]


def _dram_ap(tensor, offset, dims):
    return bass.AP(tensor, offset, dims)


def _emit_dsh_memset(nc, b):
    nc.gpsimd.memset(b.dsh[:, :, :], BIG)


def _emit_loads(nc, b, t, cam, dep):
    """DMA loads for tile t. cam/dep are host-pre-reversed for t=1, so in
    both tiles buffer row k maps to source row k (tile1 source row k is
    image row 223-k) and shifts by d in row(k)-space are source rows k - d
    for t=1 (d flips sign in reversed space: handled by the caller passing
    the per-tile d ordering)."""
    sgn = 1 if t == 0 else -1

    # dsh slot di in (0,1,2) holds D(row(k) + sgn*(di-1)); the di axis is an
    # overlapping-window read of depsrc rows (k + sgn*(di-1)), so one DMA
    # covers all (k, di) except the edge partitions that would read out of
    # bounds (those keep the BIG fill or get a small edge DMA).
    # main DMA: k in [k0m, k1m) where all three di are in range.
    dd = [sgn * (di - 1) for di in range(3)]
    k0m = max(max(0, -d) for d in dd)
    k1m = min(min(KP, H - d) for d in dd)
    nc.sync.dma_start(
        out=b.dsh[k0m:k1m, :, X0:X1],
        in_=bass.AP(dep, (k0m + dd[0]) * W,
                    [[W, k1m - k0m], [sgn * W, 3], [1, W]]))
    # edge partitions: load each valid di-slice individually (single-row
    # DMAs; avoids negative strides)
    for k in list(range(0, k0m)) + list(range(k1m, KP)):
        for di in range(3):
            if 0 <= k + dd[di] < H:
                nc.sync.dma_start(
                    out=b.dsh[k:k + 1, di, X0:X1],
                    in_=bass.AP(dep, (k + dd[di]) * W, [[0, 1], [1, W]]))


CHUNKS = [(0, 8), (8, 16), (16, 21)]


def _emit_cam_chunk(nc, b, cam, c0, c1):
    nc.sync.dma_start(
        out=b.camb[:, c0:c1, :],
        in_=bass.AP(cam, c0 * H * W, [[W, KP], [H * W, c1 - c0], [1, W]]))


def _emit_ddif(nc, b, t):
    """DVE: ddif -> dsq."""
    v = nc.vector

    # ddif rows dy=-1,0,+1: window AP over 3 x-offsets vs broadcast center.
    # ddif[k, 3*di + i, x] = dsh[k, di, x + i - 1] - dsh[k, di(center)=1, x]
    # computed for x in [1, 227).
    dst, _ = b.dsh.ap[0][0], None
    for di in range(3):
        in0 = bass.AP(b.dsh.tensor, b.dsh.offset + di * XF + 0,
                      [[dst, KP], [1, 3], [1, 226]])
        in1 = bass.AP(b.dsh.tensor, b.dsh.offset + 1 * XF + 1,
                      [[dst, KP], [0, 3], [1, 226]])
        v.tensor_tensor(out=b.ddif[:, 3 * di:3 * di + 3, 1:227],
                        in0=in0, in1=in1, op=Alu.subtract)

    # dsq = ddif^2 (bf16 2x on DVE)
    v.tensor_tensor(out=b.dsq[:, :, 1:227], in0=b.ddif[:, :, 1:227],
                    in1=b.ddif[:, :, 1:227], op=Alu.mult)


def _emit_exp(nc, b, t, bias2):
    """ACT: w~ = exp(-50*dsq + ln wspat)  (4 instrs by wspat class)."""
    s = nc.scalar
    b.bias2 = bias2
    s.activation(out=b.wb[:, 1:8:2, 1:227], in_=b.dsq[:, 1:8:2, 1:227],
                 func=Act.Exp, scale=-50.0, bias=b.bias2[:, 0:1])
    s.activation(out=b.wb[:, 0:3:2, 1:227], in_=b.dsq[:, 0:3:2, 1:227],
                 func=Act.Exp, scale=-50.0, bias=b.bias2[:, 1:2])
    s.activation(out=b.wb[:, 6:9:2, 1:227], in_=b.dsq[:, 6:9:2, 1:227],
                 func=Act.Exp, scale=-50.0, bias=b.bias2[:, 1:2])
    s.activation(out=b.wb[:, 4, 1:227], in_=b.dsq[:, 4, 1:227],
                 func=Act.Exp, scale=-50.0)


def _emit_wderiv(nc, b, t):
    """DVE: wsum, X."""
    v = nc.vector
    # wsum_m = w_m + w_{m+3} + w_{m+6}
    v.tensor_tensor(out=b.wsum[:, :, 1:227], in0=b.wb[:, 0:3, 1:227],
                    in1=b.wb[:, 3:6, 1:227], op=Alu.add)
    v.tensor_tensor(out=b.wsum[:, :, 1:227], in0=b.wsum[:, :, 1:227],
                    in1=b.wb[:, 6:9, 1:227], op=Alu.add)
    # X = wsum_0(x+1) + wsum_1(x) + wsum_2(x-1), needs k up to 112
    v.tensor_tensor(out=b.xb[:, X0:X1], in0=b.wsum[:, 0, X0 + 1:X1 + 1],
                    in1=b.wsum[:, 1, X0:X1], op=Alu.add)
    v.tensor_tensor(out=b.xb[:, X0:X1], in0=b.xb[:, X0:X1],
                    in1=b.wsum[:, 2, X0 - 1:X1 - 1], op=Alu.add)


def _emit_pool_w(nc, b, t):
    """Pool: R (from w~), Wtot (from wsum)."""
    g = nc.gpsimd
    wst = b.wb.ap[0][0]
    w_s3 = lambda q0, xoff: bass.AP(b.wb.tensor, b.wb.offset + q0 * XF + xoff,
                                    [[wst, MP], [3 * XF, 3], [1, 224]])
    g.tensor_tensor(out=b.rb[:, :, X0:X1], in0=w_s3(0, X0), in1=w_s3(1, X0),
                    op=Alu.add)
    g.tensor_tensor(out=b.rb[:, :, X0:X1], in0=b.rb[:, :, X0:X1],
                    in1=w_s3(2, X0), op=Alu.add)
    g.tensor_tensor(out=b.wtot[:, X0:X1], in0=b.wsum[0:MP, 0, X0:X1],
                    in1=b.wsum[0:MP, 1, X0:X1], op=Alu.add)
    g.tensor_tensor(out=b.wtot[:, X0:X1], in0=b.wtot[:, X0:X1],
                    in1=b.wsum[0:MP, 2, X0:X1], op=Alu.add)


def _mm(nc, wmb, out, mat_slot, rhs, start, stop, kp=KP):
    nc.tensor.matmul(out=out, lhsT=wmb[0:kp, mat_slot, :], rhs=rhs,
                     start=start, stop=stop)


def _emit_sq_act(nc, b, t):
    s = nc.scalar
    s.activation(out=b.gsq[:, 8:14, :], in_=b.camb[0:MP, 8:14, :],
                 func=Act.Square)
    s.activation(out=b.gsq[:, 0:8, :], in_=b.camb[0:MP, 0:8, :],
                 func=Act.Square)


def _emit_sq_dve(nc, b, t):
    v = nc.vector
    v.tensor_tensor(out=b.gsq[:, 14:16, :], in0=b.camb[0:MP, 14:16, :],
                    in1=b.camb[0:MP, 14:16, :], op=Alu.mult)
    v.tensor_tensor(out=b.gsq[:, 16:21, :], in0=b.camb[0:MP, 16:21, :],
                    in1=b.camb[0:MP, 16:21, :], op=Alu.mult)


def _emit_P(nc, b, t):
    v = nc.vector
    v.tensor_tensor(out=b.pb[:, :, :], in0=b.camb[0:MP, 0:7, :],
                    in1=b.camb[0:MP, 7:14, :], op=Alu.add)
    v.tensor_tensor(out=b.pb[:, :, :], in0=b.pb[:, :, :],
                    in1=b.camb[0:MP, 14:21, :], op=Alu.add)


def _emit_scam_mm(nc, b, ps, t, wmb):
    """PE shift-matmuls into the two staging banks (2 imgs per stage)."""
    sgn = 1 if t == 0 else -1
    stages = [ps.sc, ps.sc2]
    for si, i0 in enumerate(range(0, 10, 2)):
        stage = stages[si % 2]
        for j in range(2):
            dy, cp = SCAM[i0 + j]
            _mm(nc, wmb, stage[:, j, 0:W], MAT_B[sgn * dy], b.camb[:, cp, :],
                start=True, stop=True)


def _emit_scam_copy(nc, b, ps, t):
    """drain staging psum -> scamb: ACT 3 pairs, DVE 2 pairs."""
    s = nc.scalar
    v = nc.vector
    stages = [ps.sc, ps.sc2]
    for si, i0 in enumerate(range(0, 10, 2)):
        stage = stages[si % 2]
        if si < 3:
            s.activation(out=b.scamb[:, i0:i0 + 2, :],
                         in_=stage[:, :, 0:W], func=Act.Copy)
        else:
            v.tensor_copy(out=b.scamb[:, i0:i0 + 2, :],
                          in_=stage[:, :, 0:W])


def _emit_prod(nc, b, t):
    v = nc.vector
    runs = [(0, 2), (2, 3), (5, 3), (8, 3), (11, 2)]
    for r0, n in runs:
        dy, c0, cp = PRODS[r0]
        if dy == 0:
            src = b.camb[0:MP, cp:cp + n, :]
        else:
            s0 = SCAM_IDX[(dy, cp)]
            src = b.scamb[:, s0:s0 + n, :]
        v.tensor_tensor(out=b.prod[:, r0:r0 + n, :],
                        in0=b.pb[:, c0:c0 + n, :], in1=src, op=Alu.mult)


def _emit_E(nc, b, ps, t, wmb):
    e = ps.e[t]
    for c in range(C):
        _mm(nc, wmb, e[:, 0:W], MAT_B[0], b.gsq[:, c, :], start=(c == 0),
            stop=(c == C - 1), kp=MP)


def _emit_Pi(nc, b, ps, t, wmb):
    for g in range(3):
        idxs = GROUPS[g]
        for i, pi_idx in enumerate(idxs):
            _mm(nc, wmb, ps.pi(g), MAT_B[0], b.prod[:, pi_idx, :],
                start=(i == 0), stop=(i == len(idxs) - 1), kp=MP)


def _emit_om(nc, b, ps, t, wmb):
    om_mats = (MAT_DM, MAT_DT, MAT_DP) if t == 0 else (MAT_DP, MAT_DT, MAT_DM)
    for g in range(3):
        _mm(nc, wmb, ps.om(g), om_mats[g], b.xb[:, X0:X1], start=True,
            stop=True)


def _emit_amr_om(nc, b, ps, t):
    """T3 amrs (om psum freed early for the other tile)."""
    v = nc.vector
    v.affine_mul_reduce(out=b.scr[:, 0:2, :], accum_out=b.acc[:, 3:4],
                        in0=b.gsq[:, 9:11, :], in1=ps.om01[:, :, 0:W],
                        scale=3.0, bias=0.0)
    v.affine_mul_reduce(out=b.scr[:, 2, :], accum_out=b.acc[:, 4:5],
                        in0=b.gsq[:, 11, :], in1=ps.om2[:, 0:W],
                        scale=3.0, bias=0.0)


def _emit_amr_rest(nc, b, ps, t, out):
    """T1/T2 amrs -> acc -> DRAM."""
    v = nc.vector
    rst = b.rb.ap[0][0]
    v.affine_mul_reduce(out=b.scr[:, 0, :], accum_out=b.acc[:, 0:1],
                        in0=b.wtot[:, X0:X1], in1=ps.e[t][:, 0:W],
                        scale=1.0, bias=0.0)
    r_rev01 = bass.AP(b.rb.tensor, b.rb.offset + 2 * XF + X0,
                      [[rst, MP], [-XF, 2], [1, 224]])
    v.affine_mul_reduce(out=b.scr[:, 0:2, :], accum_out=b.acc[:, 1:2],
                        in0=r_rev01, in1=ps.pi01[:, :, 0:W],
                        scale=-2.0, bias=0.0)
    v.affine_mul_reduce(out=b.scr[:, 2, :], accum_out=b.acc[:, 2:3],
                        in0=b.rb[:, 0, X0:X1], in1=ps.pi2[:, 0:W],
                        scale=-2.0, bias=0.0)
    nc.sync.dma_start(out=out[t], in_=b.acc[:, :])


def build_nc():
    nc = bacc.Bacc("TRN2", target_bir_lowering=False)
    cam = nc.dram_tensor("cam", (C, H, W), BF16, kind="ExternalInput")
    dep = nc.dram_tensor("dep", (H, W), BF16, kind="ExternalInput")
    camr = nc.dram_tensor("camr", (C, H, W), BF16, kind="ExternalInput")
    depr = nc.dram_tensor("depr", (H, W), BF16, kind="ExternalInput")
    wm = nc.dram_tensor("wm", (NMAT, KP, MP), BF16, kind="ExternalInput")
    out = nc.dram_tensor("out", (2, MP, NACC), F32, kind="ExternalOutput")
    with tile.TileContext(nc) as tc:
        with tc.tile_pool(name="main", bufs=1) as pool, \
             tc.tile_pool(name="psum", bufs=1, space="PSUM") as ppool:
            wmb = pool.tile([KP, NMAT, MP], BF16, name="wmb", tag="wmb")
            bias2 = pool.tile([KP, 2], F32, name="bias2s", tag="bias2s")
            bs = {t: _T(pool, t) for t in (0, 1)}
            ps = _PS(ppool)
            b0, b1 = bs[0], bs[1]
            # Pool: memsets first
            _emit_dsh_memset(nc, b0)
            _emit_dsh_memset(nc, b1)
            nc.gpsimd.memset(bias2[:, 0:1], -0.02 * 1.0)
            nc.gpsimd.memset(bias2[:, 1:2], -0.02 * 2.0)
            # DMA queue: dsh first, then wm, then cam chunks (c1 first: it
            # feeds scam/prod/sq), then the rest
            _emit_loads(nc, b0, 0, cam, dep)
            _emit_loads(nc, b1, 1, camr, depr)
            nc.sync.dma_start(
                out=wmb[:, :, :],
                in_=bass.AP(wm, 0, [[MP, KP], [KP * MP, NMAT], [1, MP]]))
            for (c0, c1), tt in (((8, 16), 0), ((8, 16), 1), ((0, 8), 0),
                                 ((16, 21), 0), ((0, 8), 1), ((16, 21), 1)):
                _emit_cam_chunk(nc, bs[tt], cam if tt == 0 else camr, c0, c1)
            # DVE: depth diffs for both tiles up front
            _emit_ddif(nc, b0, 0)
            _emit_ddif(nc, b1, 1)
            # ACT: exp as soon as dsq lands; squares between
            _emit_exp(nc, b0, 0, bias2)
            _emit_sq_act(nc, b0, 0)
            _emit_exp(nc, b1, 1, bias2)
            _emit_sq_act(nc, b1, 1)
            # DVE: w-derivations, squares, P
            _emit_wderiv(nc, b0, 0)
            _emit_wderiv(nc, b1, 1)
            # Pool: R/Wtot
            _emit_pool_w(nc, b0, 0)
            _emit_pool_w(nc, b1, 1)
            # PE: scam t0, om t0, scam t1, E t0 ...
            _emit_scam_mm(nc, b0, ps, 0, wmb)
            _emit_om(nc, b0, ps, 0, wmb)
            _emit_scam_copy(nc, b0, ps, 0)
            _emit_sq_dve(nc, b0, 0)
            _emit_P(nc, b0, 0)
            _emit_prod(nc, b0, 0)
            _emit_amr_om(nc, b0, ps, 0)
            _emit_E(nc, b0, ps, 0, wmb)
            _emit_Pi(nc, b0, ps, 0, wmb)
            _emit_scam_mm(nc, b1, ps, 1, wmb)
            _emit_scam_copy(nc, b1, ps, 1)
            _emit_sq_dve(nc, b1, 1)
            _emit_P(nc, b1, 1)
            _emit_amr_rest(nc, b0, ps, 0, out)
            _emit_prod(nc, b1, 1)
            _emit_om(nc, b1, ps, 1, wmb)
            _emit_amr_om(nc, b1, ps, 1)
            _emit_E(nc, b1, ps, 1, wmb)
            _emit_Pi(nc, b1, ps, 1, wmb)
            _emit_amr_rest(nc, b1, ps, 1, out)
    nc.finalize()
    return nc


_CACHE = {}


def _get_nc():
    if "nc" not in _CACHE:
        _CACHE["nc"] = build_nc()
    return _CACHE["nc"]


def _run(in_maps, **kw):
    return run_bass_kernel_spmd(_get_nc(), in_maps, core_ids=list(range(N)), **kw)


def _make_in_maps(cam_map, depth_map):
    import ml_dtypes
    camb = np.ascontiguousarray(cam_map, dtype=np.float32).astype(ml_dtypes.bfloat16)
    dep = np.ascontiguousarray(depth_map, dtype=np.float32).astype(ml_dtypes.bfloat16)
    cambr = np.ascontiguousarray(camb[:, :, ::-1, :])
    depre = np.ascontiguousarray(dep[:, :, ::-1, :])
    wm = _build_wm().astype(ml_dtypes.bfloat16)
    return [{"cam": camb[i], "dep": dep[i, 0], "camr": cambr[i],
             "depr": depre[i, 0], "wm": wm} for i in range(N)]


def kernel(cam_map, depth_map):
    r = _run(_make_in_maps(cam_map, depth_map))
    tot = sum(float(m["out"].astype(np.float64).sum()) for m in r.results)
    return np.array(tot / (N * H * W), dtype=np.float32)


# revision 13
# speedup vs baseline: 2.4090x; 1.1324x over previous
"""DepthConsistencyLoss Trainium2 kernel (8 NeuronCores, batch-parallel).

loss = mean_{n,l} sum_{r=0..188} w_{r%9}[l] * (cam_unfold[r,l] - cam_center[r%21,l])^2

Restructure (loss*N*H*W = sum_n T1 - 2*T2 + 3*T3'):
  Key identity: S_{-dp} w_p = w_{8-p} (spatial weights symmetric), so with
  masked weights w~_q = w_q * [l + dq inside image]:
    T1 = sum_l E * W~tot            E = sum_c cam_c^2, W~tot = sum_q w~_q
    T2 = sum_g sum_l R~_{2-g} * Pi_g   R~_h = row sums of w~,
         Pi_g = sum_{c0} P_c0 * S_{(dy,0)} cam_{c'}  (13 distinct products)
    T3' = sum_{c'} G_{c'} * Om_{c'}    Om from vertical band-matmuls of
         X = S_(0,1)wsum_0 + wsum_1 + S_(0,-1)wsum_2
  Masking is free: the depth windows are host-padded with BIG=1e4, so
  invalid-shift weights come out exp(-50*BIG^2) = 0.

Layout: 2 tiles of 114 partitions = image rows; tile0 rows k=0..113,
tile1 REVERSED rows 223-k (so per-tile outputs m=0..111 start at partition 0
on both tiles). y-shifts run on the otherwise idle PE with [114,112]
shift/band matrices; x-shifts are free-dim AP offsets. The host pre-packs
per-tile partition-major arrays (bf16) so each DMA descriptor moves >=512B
contiguous (avoids the small-descriptor bandwidth penalty).
Each core does one batch element; host sums the 8 x [2,112,5] partials.
"""

import os
import sys

import numpy as np

for _p in ("/opt/trn_rl_repo", os.path.expanduser("~/.axon_site/_ro/trn_rl_repo")):
    if os.path.isdir(_p) and _p not in sys.path:
        sys.path.insert(0, _p)

import concourse.bass as bass
import concourse.bacc as bacc
import concourse.tile as tile
from concourse import mybir
from concourse.bass_utils import run_bass_kernel_spmd

F32 = mybir.dt.float32
BF16 = mybir.dt.bfloat16
Alu = mybir.AluOpType
Act = mybir.ActivationFunctionType

N, C, H, W = 8, 21, 224, 224
KP = 114          # k-space partitions per tile (rows + 2 halo for PE shifts)
MP = 112          # m-space output rows per tile
XF = 228          # padded depth row: [2 pad][224][2 pad]
X0, X1 = 2, 226
NACC = 5
BIG = 1.0e4

# 13 distinct products (dy, c0, c'), dy-major; runs of consecutive c0/c'.
PRODS = [(-2, 0, 10), (-2, 1, 11),
         (-1, 4, 9), (-1, 5, 10), (-1, 6, 11),
         (0, 2, 9), (0, 3, 10), (0, 4, 11),
         (1, 0, 9), (1, 1, 10), (1, 2, 11),
         (2, 5, 9), (2, 6, 10)]
GROUPS = {0: [8, 9, 10, 6, 7, 11, 12],
          1: [8, 9, 5, 6, 7, 3, 4],
          2: [0, 1, 5, 6, 2, 3, 4]}
SCAM = [(dy, cp) for (dy, c0, cp) in PRODS if dy != 0]   # 10 images
SCAM_IDX = {s: i for i, s in enumerate(SCAM)}

MAT_B = {dy: 2 + dy for dy in (-2, -1, 0, 1, 2)}   # B_j at slot 2+j
MAT_DM, MAT_DT, MAT_DP = 5, 6, 7                   # B0+B-1, tri, B0+B+1
NMAT = 8


def _build_wm():
    """[KP, NMAT, MP] bf16 shift/band matrices W[k, m] (partition-major)."""
    wm = np.zeros((NMAT, KP, MP), np.float32)
    for j in (-2, -1, 0, 1, 2):
        for m in range(MP):
            k = m + j
            if 0 <= k < KP:
                wm[MAT_B[j], k, m] = 1.0
    wm[MAT_DM] = wm[MAT_B[0]] + wm[MAT_B[-1]]
    wm[MAT_DT] = wm[MAT_B[-1]] + wm[MAT_B[0]] + wm[MAT_B[1]]
    wm[MAT_DP] = wm[MAT_B[0]] + wm[MAT_B[1]]
    return np.ascontiguousarray(wm.transpose(1, 0, 2))


class _T:
    """Per-tile SBUF buffers."""

    def __init__(self, pool, t):
        self.t = t
        self.camb = pool.tile([KP, C, W], BF16, name=f"camb{t}", tag=f"camb{t}")
        self.dsh = pool.tile([KP, 3, XF], BF16, name=f"dsh{t}", tag=f"dsh{t}")
        self.gsq = pool.tile([MP, C, W], BF16, name=f"gsq{t}", tag=f"gsq{t}")
        self.ddif = pool.tile([KP, 9, XF], BF16, name=f"ddif{t}", tag=f"ddif{t}")
        self.dsq = pool.tile([KP, 9, XF], BF16, name=f"dsq{t}", tag=f"dsq{t}")
        self.wb = pool.tile([KP, 9, XF], BF16, name=f"wb{t}", tag=f"wb{t}")
        self.wsum = pool.tile([KP, 3, XF], BF16, name=f"wsum{t}", tag=f"wsum{t}")
        self.xb = pool.tile([KP, XF], BF16, name=f"xb{t}", tag=f"xb{t}")
        self.wtot = pool.tile([MP, XF], BF16, name=f"wtot{t}", tag=f"wtot{t}")
        self.rb = pool.tile([MP, 3, XF], BF16, name=f"rb{t}", tag=f"rb{t}")
        self.pb = pool.tile([MP, 7, W], BF16, name=f"pb{t}", tag=f"pb{t}")
        self.scamb = pool.tile([MP, 10, W], BF16, name=f"scamb{t}", tag=f"scamb{t}")
        self.prod = pool.tile([MP, 13, W], BF16, name=f"prod{t}", tag=f"prod{t}")
        self.scr = pool.tile([MP, 3, W], BF16, name=f"scr{t}", tag=f"scr{t}")
        self.acc = pool.tile([MP, NACC], F32, name=f"acc{t}", tag=f"acc{t}")


class _PS:
    """Shared PSUM tiles, one bank each (stride-256 keeps matmul outs
    in-bank); E double-buffered per tile, the rest reused (WAR syncs)."""

    def __init__(self, ppool):
        self.e = {0: ppool.tile([MP, 256], F32, name="e0", tag="e0"),
                  1: ppool.tile([MP, 256], F32, name="e1", tag="e1")}
        self.pi01 = ppool.tile([MP, 2, 256], F32, name="pi01", tag="pi01")
        self.pi2 = ppool.tile([MP, 256], F32, name="pi2", tag="pi2")
        self.om01 = ppool.tile([MP, 2, 256], F32, name="om01", tag="om01")
        self.om2 = ppool.tile([MP, 256], F32, name="om2", tag="om2")
        self.sc = ppool.tile([MP, 2, 256], F32, name="sc", tag="sc")
        self.sc2 = ppool.tile([MP, 2, 256], F32, name="sc2", tag="sc2")

    def pi(self, g):
        return self.pi01[:, g, 0:W] if g < 2 else self.pi2[:, 0:W]

    def om(self, g):
        return self.om01[:, g, 0:W] if g < 2 else self.om2[:, 0:W]


def _emit_cam_chunk(nc, b, ct, c0, c1):
    """camb[:, c0:c1, :] from the per-tile partition-major DRAM image."""
    nc.sync.dma_start(
        out=b.camb[:, c0:c1, :],
        in_=bass.AP(ct, c0 * W, [[C * W, KP], [W, c1 - c0], [1, W]]))


def _emit_dsh_load(nc, b, dsh):
    nc.sync.dma_start(
        out=b.dsh[:, :, :],
        in_=bass.AP(dsh, 0, [[3 * XF, KP], [XF, 3], [1, XF]]))


def _emit_ddif(nc, b, t):
    """DVE: ddif -> dsq (bf16 2x)."""
    v = nc.vector
    dst = b.dsh.ap[0][0]
    for di in range(3):
        in0 = bass.AP(b.dsh.tensor, b.dsh.offset + di * XF + 0,
                      [[dst, KP], [1, 3], [1, 226]])
        in1 = bass.AP(b.dsh.tensor, b.dsh.offset + 1 * XF + 1,
                      [[dst, KP], [0, 3], [1, 226]])
        v.tensor_tensor(out=b.ddif[:, 3 * di:3 * di + 3, 1:227],
                        in0=in0, in1=in1, op=Alu.subtract)
    v.tensor_tensor(out=b.dsq[:, :, 1:227], in0=b.ddif[:, :, 1:227],
                    in1=b.ddif[:, :, 1:227], op=Alu.mult)


def _emit_exp(nc, b, t, bias2):
    """ACT: w~ = exp(-50*dsq + ln wspat)  (4 instrs by wspat class)."""
    s = nc.scalar
    s.activation(out=b.wb[:, 1:8:2, 1:227], in_=b.dsq[:, 1:8:2, 1:227],
                 func=Act.Exp, scale=-50.0, bias=bias2[:, 0:1])
    s.activation(out=b.wb[:, 0:3:2, 1:227], in_=b.dsq[:, 0:3:2, 1:227],
                 func=Act.Exp, scale=-50.0, bias=bias2[:, 1:2])
    s.activation(out=b.wb[:, 6:9:2, 1:227], in_=b.dsq[:, 6:9:2, 1:227],
                 func=Act.Exp, scale=-50.0, bias=bias2[:, 1:2])
    s.activation(out=b.wb[:, 4, 1:227], in_=b.dsq[:, 4, 1:227],
                 func=Act.Exp, scale=-50.0)


def _emit_wderiv(nc, b, t):
    """DVE: wsum, X."""
    v = nc.vector
    v.tensor_tensor(out=b.wsum[:, :, 1:227], in0=b.wb[:, 0:3, 1:227],
                    in1=b.wb[:, 3:6, 1:227], op=Alu.add)
    v.tensor_tensor(out=b.wsum[:, :, 1:227], in0=b.wsum[:, :, 1:227],
                    in1=b.wb[:, 6:9, 1:227], op=Alu.add)
    v.tensor_tensor(out=b.xb[:, X0:X1], in0=b.wsum[:, 0, X0 + 1:X1 + 1],
                    in1=b.wsum[:, 1, X0:X1], op=Alu.add)
    v.tensor_tensor(out=b.xb[:, X0:X1], in0=b.xb[:, X0:X1],
                    in1=b.wsum[:, 2, X0 - 1:X1 - 1], op=Alu.add)


def _emit_pool_w(nc, b, t):
    """Pool: R (from w~), Wtot (from wsum)."""
    g = nc.gpsimd
    wst = b.wb.ap[0][0]
    w_s3 = lambda q0, xoff: bass.AP(b.wb.tensor, b.wb.offset + q0 * XF + xoff,
                                    [[wst, MP], [3 * XF, 3], [1, 224]])
    g.tensor_tensor(out=b.rb[:, :, X0:X1], in0=w_s3(0, X0), in1=w_s3(1, X0),
                    op=Alu.add)
    g.tensor_tensor(out=b.rb[:, :, X0:X1], in0=b.rb[:, :, X0:X1],
                    in1=w_s3(2, X0), op=Alu.add)
    g.tensor_tensor(out=b.wtot[:, X0:X1], in0=b.wsum[0:MP, 0, X0:X1],
                    in1=b.wsum[0:MP, 1, X0:X1], op=Alu.add)
    g.tensor_tensor(out=b.wtot[:, X0:X1], in0=b.wtot[:, X0:X1],
                    in1=b.wsum[0:MP, 2, X0:X1], op=Alu.add)


def _mm(nc, wmb, out, mat_slot, rhs, start, stop, kp=KP):
    nc.tensor.matmul(out=out, lhsT=wmb[0:kp, mat_slot, :], rhs=rhs,
                     start=start, stop=stop)


def _emit_sq_act(nc, b, t, rng):
    nc.scalar.activation(out=b.gsq[:, rng[0]:rng[1], :],
                         in_=b.camb[0:MP, rng[0]:rng[1], :], func=Act.Square)


def _emit_sq_dve(nc, b, t, rng):
    nc.vector.tensor_tensor(out=b.gsq[:, rng[0]:rng[1], :],
                            in0=b.camb[0:MP, rng[0]:rng[1], :],
                            in1=b.camb[0:MP, rng[0]:rng[1], :], op=Alu.mult)


def _emit_P1(nc, b, t):
    """P partial: cam[0:7] + cam[7:14] (chunks A+B)."""
    nc.vector.tensor_tensor(out=b.pb[:, :, :], in0=b.camb[0:MP, 0:7, :],
                            in1=b.camb[0:MP, 7:14, :], op=Alu.add)


def _emit_P2(nc, b, t):
    """P += cam[14:21] (chunk C)."""
    nc.vector.tensor_tensor(out=b.pb[:, :, :], in0=b.pb[:, :, :],
                            in1=b.camb[0:MP, 14:21, :], op=Alu.add)


def _emit_scam_mm(nc, b, ps, t, wmb):
    sgn = 1 if t == 0 else -1
    stages = [ps.sc, ps.sc2]
    for si, i0 in enumerate(range(0, 10, 2)):
        stage = stages[si % 2]
        for j in range(2):
            dy, cp = SCAM[i0 + j]
            _mm(nc, wmb, stage[:, j, 0:W], MAT_B[sgn * dy], b.camb[:, cp, :],
                start=True, stop=True)


def _emit_scam_copy(nc, b, ps, t):
    """drain staging psum -> scamb: ACT 3 pairs, DVE 2 pairs."""
    stages = [ps.sc, ps.sc2]
    for si, i0 in enumerate(range(0, 10, 2)):
        stage = stages[si % 2]
        if si < 3:
            nc.scalar.activation(out=b.scamb[:, i0:i0 + 2, :],
                                 in_=stage[:, :, 0:W], func=Act.Copy)
        else:
            nc.vector.tensor_copy(out=b.scamb[:, i0:i0 + 2, :],
                                  in_=stage[:, :, 0:W])


def _emit_prod(nc, b, t):
    v = nc.vector
    runs = [(0, 2), (2, 3), (5, 3), (8, 3), (11, 2)]
    for r0, n in runs:
        dy, c0, cp = PRODS[r0]
        if dy == 0:
            src = b.camb[0:MP, cp:cp + n, :]
        else:
            s0 = SCAM_IDX[(dy, cp)]
            src = b.scamb[:, s0:s0 + n, :]
        v.tensor_tensor(out=b.prod[:, r0:r0 + n, :],
                        in0=b.pb[:, c0:c0 + n, :], in1=src, op=Alu.mult)


def _emit_E(nc, b, ps, t, wmb, crng, first=False):
    c0, c1 = crng
    for c in range(c0, c1):
        _mm(nc, wmb, ps.e[t][:, 0:W], MAT_B[0], b.gsq[:, c, :],
            start=(first and c == c0), stop=(c == C - 1), kp=MP)


def _emit_Pi(nc, b, ps, t, wmb):
    for g in range(3):
        idxs = GROUPS[g]
        for i, pi_idx in enumerate(idxs):
            _mm(nc, wmb, ps.pi(g), MAT_B[0], b.prod[:, pi_idx, :],
                start=(i == 0), stop=(i == len(idxs) - 1), kp=MP)


def _emit_om(nc, b, ps, t, wmb):
    om_mats = (MAT_DM, MAT_DT, MAT_DP) if t == 0 else (MAT_DP, MAT_DT, MAT_DM)
    for g in range(3):
        _mm(nc, wmb, ps.om(g), om_mats[g], b.xb[:, X0:X1], start=True,
            stop=True)


def _emit_amr_om(nc, b, ps, t):
    """T3 amrs (frees om psum early for the other tile)."""
    v = nc.vector
    v.affine_mul_reduce(out=b.scr[:, 0:2, :], accum_out=b.acc[:, 3:4],
                        in0=b.gsq[:, 9:11, :], in1=ps.om01[:, :, 0:W],
                        scale=3.0, bias=0.0)
    v.affine_mul_reduce(out=b.scr[:, 2, :], accum_out=b.acc[:, 4:5],
                        in0=b.gsq[:, 11, :], in1=ps.om2[:, 0:W],
                        scale=3.0, bias=0.0)


def _emit_amr_T1(nc, b, ps, t):
    nc.vector.affine_mul_reduce(out=b.scr[:, 0, :], accum_out=b.acc[:, 0:1],
                                in0=b.wtot[:, X0:X1], in1=ps.e[t][:, 0:W],
                                scale=1.0, bias=0.0)


def _emit_amr_T2(nc, b, ps, t, out):
    v = nc.vector
    rst = b.rb.ap[0][0]
    r_rev01 = bass.AP(b.rb.tensor, b.rb.offset + 2 * XF + X0,
                      [[rst, MP], [-XF, 2], [1, 224]])
    v.affine_mul_reduce(out=b.scr[:, 0:2, :], accum_out=b.acc[:, 1:2],
                        in0=r_rev01, in1=ps.pi01[:, :, 0:W],
                        scale=-2.0, bias=0.0)
    v.affine_mul_reduce(out=b.scr[:, 2, :], accum_out=b.acc[:, 2:3],
                        in0=b.rb[:, 0, X0:X1], in1=ps.pi2[:, 0:W],
                        scale=-2.0, bias=0.0)
    nc.sync.dma_start(out=out[t], in_=b.acc[:, :])


def build_nc():
    nc = bacc.Bacc("TRN2", target_bir_lowering=False)
    ct = {0: nc.dram_tensor("ct0", (KP, C, W), BF16, kind="ExternalInput"),
          1: nc.dram_tensor("ct1", (KP, C, W), BF16, kind="ExternalInput")}
    dsh = {0: nc.dram_tensor("dsh0", (KP, 3, XF), BF16, kind="ExternalInput"),
           1: nc.dram_tensor("dsh1", (KP, 3, XF), BF16, kind="ExternalInput")}
    wm = nc.dram_tensor("wm", (KP, NMAT, MP), BF16, kind="ExternalInput")
    out = nc.dram_tensor("out", (2, MP, NACC), F32, kind="ExternalOutput")
    with tile.TileContext(nc) as tc:
        with tc.tile_pool(name="main", bufs=1) as pool, \
             tc.tile_pool(name="psum", bufs=1, space="PSUM") as ppool:
            wmb = pool.tile([KP, NMAT, MP], BF16, name="wmb", tag="wmb")
            bias2 = pool.tile([KP, 2], F32, name="bias2s", tag="bias2s")
            warm = pool.tile([KP, 2], BF16, name="warm", tag="warm")
            bs = {t: _T(pool, t) for t in (0, 1)}
            ps = _PS(ppool)
            b0, b1 = bs[0], bs[1]

            # Pool: bias consts; ACT: warm up the Exp table load
            nc.gpsimd.memset(bias2[:, 0:1], -0.02 * 1.0)
            nc.gpsimd.memset(bias2[:, 1:2], -0.02 * 2.0)
            nc.scalar.activation(out=warm[:, :], in_=bias2[:, :],
                                 func=Act.Exp)

            # DMA queue: dsh first, wm (for scam), then cam chunks B, A, C
            _emit_dsh_load(nc, b0, dsh[0])
            _emit_dsh_load(nc, b1, dsh[1])
            nc.sync.dma_start(
                out=wmb[:, :, :],
                in_=bass.AP(wm, 0, [[NMAT * MP, KP], [MP, NMAT], [1, MP]]))
            for (c0, c1), tt in (((7, 14), 0), ((7, 14), 1), ((0, 7), 0),
                                 ((0, 7), 1), ((14, 21), 0), ((14, 21), 1)):
                _emit_cam_chunk(nc, bs[tt], ct[tt], c0, c1)

            # DVE: depth diffs both tiles up front
            _emit_ddif(nc, b0, 0)
            _emit_ddif(nc, b1, 1)
            # ACT: exp asap; squares of chunks B/A between
            _emit_exp(nc, b0, 0, bias2)
            _emit_sq_act(nc, b0, 0, (7, 14))
            _emit_exp(nc, b1, 1, bias2)
            _emit_sq_act(nc, b1, 1, (7, 14))
            _emit_sq_act(nc, b0, 0, (0, 7))
            _emit_sq_act(nc, b1, 1, (0, 7))
            # DVE: w-derivations
            _emit_wderiv(nc, b0, 0)
            _emit_wderiv(nc, b1, 1)
            # Pool: R/Wtot
            _emit_pool_w(nc, b0, 0)
            _emit_pool_w(nc, b1, 1)
            # PE: scam (chunk B), om (after X)
            _emit_scam_mm(nc, b0, ps, 0, wmb)
            _emit_om(nc, b0, ps, 0, wmb)
            _emit_scam_copy(nc, b0, ps, 0)
            _emit_scam_mm(nc, b1, ps, 1, wmb)
            _emit_P1(nc, b0, 0)
            _emit_amr_om(nc, b0, ps, 0)
            _emit_om(nc, b1, ps, 1, wmb)   # om psum freed by amr_om t0
            _emit_scam_copy(nc, b1, ps, 1)
            _emit_P1(nc, b1, 1)
            _emit_amr_om(nc, b1, ps, 1)
            # E accumulation: B and A channel ranges first
            _emit_E(nc, b0, ps, 0, wmb, (7, 14), first=True)
            _emit_E(nc, b0, ps, 0, wmb, (0, 7))
            _emit_E(nc, b1, ps, 1, wmb, (7, 14), first=True)
            _emit_E(nc, b1, ps, 1, wmb, (0, 7))
            # chunk C arrives: squares (DVE), P complete, products, Pi
            _emit_sq_dve(nc, b0, 0, (14, 21))
            _emit_P2(nc, b0, 0)
            _emit_prod(nc, b0, 0)
            _emit_sq_dve(nc, b1, 1, (14, 21))
            _emit_P2(nc, b1, 1)
            _emit_E(nc, b0, ps, 0, wmb, (14, 21))
            _emit_Pi(nc, b0, ps, 0, wmb)
            _emit_amr_T1(nc, b0, ps, 0)
            _emit_prod(nc, b1, 1)
            _emit_amr_T2(nc, b0, ps, 0, out)
            _emit_E(nc, b1, ps, 1, wmb, (14, 21))
            _emit_Pi(nc, b1, ps, 1, wmb)
            _emit_amr_T1(nc, b1, ps, 1)
            _emit_amr_T2(nc, b1, ps, 1, out)
    nc.finalize()
    return nc


_CACHE = {}


def _get_nc():
    if "nc" not in _CACHE:
        _CACHE["nc"] = build_nc()
    return _CACHE["nc"]


def _run(in_maps, **kw):
    return run_bass_kernel_spmd(_get_nc(), in_maps, core_ids=list(range(N)), **kw)


def _make_in_maps(cam_map, depth_map):
    import ml_dtypes

    BF = ml_dtypes.bfloat16
    camb = np.ascontiguousarray(cam_map, dtype=np.float32).astype(BF)
    dep = np.ascontiguousarray(depth_map, dtype=np.float32)[:, 0]  # (N,H,W)

    # per-tile partition-major cam: tile0 rows 0..113, tile1 rows 223..110
    ct0 = np.ascontiguousarray(camb[:, :, 0:KP, :].transpose(0, 2, 1, 3))
    ct1 = np.ascontiguousarray(
        camb[:, :, H - 1:H - 1 - KP:-1, :].transpose(0, 2, 1, 3))

    # depth shift-windows with BIG padding (rows out of image and x pads)
    def build_dsh(rows):                       # rows: length-KP image rows
        out = np.full((N, KP, 3, XF), BIG, np.float32)
        for ki, r in enumerate(rows):
            for di, d in enumerate((-1, 0, 1)):
                rr = r + d
                if 0 <= rr < H:
                    out[:, ki, di, X0:X1] = dep[:, rr, :]
        return out.astype(BF)

    dsh0 = build_dsh(list(range(0, KP)))
    dsh1 = build_dsh(list(range(H - 1, H - 1 - KP, -1)))
    wmh = _build_wm().astype(BF)
    return [{"ct0": ct0[i], "ct1": ct1[i], "dsh0": dsh0[i], "dsh1": dsh1[i],
             "wm": wmh} for i in range(N)]


def kernel(cam_map, depth_map):
    r = _run(_make_in_maps(cam_map, depth_map))
    tot = sum(float(m["out"].astype(np.float64).sum()) for m in r.results)
    return np.array(tot / (N * H * W), dtype=np.float32)
